# revision 12
# baseline (speedup 1.0000x reference)
"""Trainium2 Bass kernel for nn_DeepTropNet (dense tropical transformer).

Strategy:
- Batch-parallel across cores (B=4 -> cores 0..3; cores 4..7 duplicate).
- Residual stream kept TRANSPOSED in SBUF: hT [D=128 partitions, L=197 free],
  so every projection is a natural TensorE matmul (contraction on partitions).
- All tropical (max-plus) contractions use the log-sum-exp trick at low
  temperature: max_i(a_i+b_i) = T*log(sum_i e^{a_i/T} e^{b_i/T}), separable ->
  a TensorE matmul of elementwise exponentials. Stabilizers are data-derived
  maxima (host-side for weights, on-device for activations).
- All weight transposes/exp-tables/bias-folds are host-side numpy prep.
- Custom DVE ops (affine_then_max/min) fuse the piecewise-linear fold.
"""
import sys

sys.path.insert(0, "/opt/trn_rl_repo")

import numpy as np

import concourse.bass as bass
import concourse.tile as tile
from concourse import bacc, mybir
from concourse import bass_utils

FP = mybir.dt.float32
AX = mybir.AxisListType
OP = mybir.AluOpType
ACTF = mybir.ActivationFunctionType

NL, D, H, DK, L, F, P, NC, PS = 2, 128, 8, 16, 197, 256, 8, 1000, 16
EPS = 1e-5
SCALE = DK ** -0.5
T1 = 0.01        # temperature for D=128-contraction tropical matmuls (Q,K,z)
T2 = 0.06        # temperature for DK=16-contraction tropical scores
NPATCH = 196
KCH = 6          # 768/128 contraction chunks for patch embed
KT0, KT1 = 128, L - 128   # score k-tiles


# ---------------------------------------------------------------- custom DVE ops
def _make_op(name, body_fn, reference):
    from concourse.dve_spec import Spec, lower, _has_src1
    from concourse.dve_uop import DveOpSpec
    import concourse.dve_ops as dve_ops

    for o in dve_ops.OPS:
        if o.name == name:
            return o
    spec = Spec(body=body_fn(), reference=reference)
    row = dve_ops._CUSTOM_DVE_ROW_BASE + len(dve_ops.OPS)
    assert row < 0x20
    dve_ops._SUB_OPCODE_FOR_NAME[name] = row
    shas = {}
    for ver in ("v3", "v4"):
        try:
            uops = lower(spec, ver=ver)
            shas[ver] = DveOpSpec(name=name, opcode=row, uops=uops,
                                  rd1_en=_has_src1(spec)).sha(ver)
        except Exception:
            pass
    op = dve_ops.DveOp(name, spec, subdim=False, uops_sha=shas)
    dve_ops.OPS.append(op)
    dve_ops.CUSTOM_DVE_SPECS[name] = spec
    return op


def _register_ops():
    from concourse.dve_spec import C0, C1, Src0, Src1, maxx, minn

    aff_max = _make_op(
        "ANT_AFFINE_THEN_MAX",
        lambda: maxx(Src0 * C0 + C1, Src1),
        lambda in0, in1, s0, s1, imm2: np.maximum(
            (in0.astype(np.float32) * s0 + s1), in1).astype(np.float32),
    )
    aff_min = _make_op(
        "ANT_AFFINE_THEN_MIN",
        lambda: minn(Src0 * C0 + C1, Src1),
        lambda in0, in1, s0, s1, imm2: np.minimum(
            (in0.astype(np.float32) * s0 + s1), in1).astype(np.float32),
    )
    return aff_max, aff_min


AFF_MAX, AFF_MIN = _register_ops()


# ---------------------------------------------------------------- host-side prep
class _Pack:
    def __init__(self):
        self.cols = {}
        self.n = 0

    def add(self, name, ncols):
        self.cols[name] = (self.n, ncols)
        self.n += ncols

    def sl(self, name):
        return self.cols[name]


def _layout():
    pk = _Pack()
    pk.add("xp", KCH * NPATCH)        # per-batch patch data chunks
    pk.add("b0", L)
    pk.add("pw", KCH * 128)
    pk.add("onescol", 1)
    pk.add("meancol", 1)              # 1/128
    pk.add("e8", 8 * 8)               # unit-column blocks for sigma matmuls
    pk.add("segg", 2 * 128)           # per-group head segment lhsT (rows 0:8)
    pk.add("oneh", 8 * 128)           # row-h all-ones lhsT blocks (rows 0:8)
    pk.add("onesrow", 128)            # all ones; row 0 used as [1,128] lhsT
    pk.add("epscol", 1)
    pk.add("clampcol", 1)
    pk.add("zerocol", 1)
    pk.add("padneg", 1)
    for i in range(NL):
        s = f"_{i}"
        pk.add("ln1wb" + s, 2)
        pk.add("ln2wb" + s, 2)
        pk.add("wqexpT" + s, 256)
        pk.add("cq" + s, 2)
        pk.add("wkexpT" + s, 256)
        pk.add("ck" + s, 2)
        pk.add("wvT" + s, 256)
        pk.add("gwT" + s, 8)
        pk.add("gb8" + s, 1)          # rows 0:8
        pk.add("st8" + s, 1)          # rows 0:8: scale*T2/temp_h
        pk.add("svec" + s, 2)         # per-group scale/temp cols (pad rows 0)
        pk.add("woT" + s, 256)
        pk.add("bo2" + s, 1)
        pk.add("tuexpT" + s, F)
        pk.add("ctu" + s, 2)
        pk.add("lfa" + s, 16)         # col t*8+p
        pk.add("lfc" + s, 16)
        pk.add("glc" + s, 2)
        pk.add("glc1m" + s, 2)
        pk.add("cuT" + s, F)
        pk.add("cub" + s, 2)
        pk.add("fgT" + s, F)
        pk.add("fgb" + s, 2)
        pk.add("dnT" + s, 2 * 128)
        pk.add("dnb" + s, 1)
    pk.add("fnwb", 2)
    pk.add("hwT", 1024)
    pk.add("hb", 8)
    return pk


PK = _layout()
NCOL = PK.n


def _prep_pack(inp):
    W = np.zeros((128, NCOL), np.float32)

    def put(name, arr):
        o, n = PK.sl(name)
        a = np.asarray(arr, np.float32)
        assert a.ndim == 2 and a.shape[1] == n, (name, a.shape, n)
        W[: a.shape[0], o:o + n] = a

    bn_s = inp["bn_gamma"] / (inp["bn_run_range"] + EPS)
    bn_b = inp["bn_beta"] - inp["bn_run_max"] * bn_s
    pos = inp["pos_embed"][0]                                     # [L, D]
    clsb = inp["cls_token"].reshape(D)
    b0 = np.empty((D, L), np.float32)
    b0[:, 0] = bn_s * (clsb + pos[0]) + bn_b
    b0[:, 1:] = (bn_s[:, None] * (inp["patch_b"][:, None] + pos[1:].T)
                 + bn_b[:, None])
    put("b0", b0)

    pwT = (bn_s[:, None] * inp["patch_w"]).T                      # [768, D]
    put("pw", np.concatenate([pwT[128 * k:128 * (k + 1)] for k in range(KCH)], 1))

    put("onescol", np.ones((128, 1)))
    put("meancol", np.full((128, 1), 1.0 / D))
    e8 = np.zeros((128, 64), np.float32)
    for h in range(H):
        e8[:, 8 * h + h] = 1.0
    put("e8", e8)
    segg = np.zeros((8, 2 * 128), np.float32)
    for g in range(2):
        for j in range(4):
            segg[4 * g + j, 128 * g + 32 * j:128 * g + 32 * j + DK] = 1.0
    put("segg", segg)
    oneh = np.zeros((8, 8 * 128), np.float32)
    for h in range(H):
        oneh[h, 128 * h:128 * (h + 1)] = 1.0
    put("oneh", oneh)
    put("onesrow", np.ones((128, 128)))
    put("epscol", np.full((128, 1), EPS))
    put("clampcol", np.full((128, 1), 1e-30))
    put("zerocol", np.zeros((128, 1)))
    padneg = np.zeros((128, 1), np.float32)
    for j in range(4):
        padneg[32 * j + DK:32 * (j + 1)] = -1e5
    put("padneg", padneg)

    def pad32_rows(vec):
        # [D] head-indexed (16h+d) -> two [128] group columns at rows 32j+d
        out = np.zeros((2, 128), np.float32)
        for h in range(H):
            g, j = divmod(h, 4)
            out[g, 32 * j:32 * j + DK] = vec[DK * h:DK * (h + 1)]
        return out

    for i in range(NL):
        s = f"_{i}"
        put("ln1wb" + s, np.stack([inp["n1_w"][i], inp["n1_b"][i]], 1))
        put("ln2wb" + s, np.stack([inp["n2_w"][i], inp["n2_b"][i]], 1))
        for nm, bkey in (("wq", "bq"), ("wk", "bk")):
            Wt = inp[nm][i]
            mW = Wt.max(1)
            we = np.exp((Wt - mW[:, None]) / T1).T          # [i, o=16h+d]
            wep = np.zeros((128, 256), np.float32)
            for h in range(H):
                g, j = divmod(h, 4)
                wep[:, 128 * g + 32 * j:128 * g + 32 * j + DK] = \
                    we[:, DK * h:DK * (h + 1)]
            put(nm + "expT" + s, wep)
            put("c" + nm[1] + s, pad32_rows(mW + inp[bkey][i]).T)
        wvp = np.zeros((128, 256), np.float32)
        wvT_ = inp["wv"][i].T                               # [i, 16h+d]
        for h in range(H):
            g, j = divmod(h, 4)
            wvp[:, 128 * g + 32 * j:128 * g + 32 * j + DK] = \
                wvT_[:, DK * h:DK * (h + 1)]
        put("wvT" + s, wvp)
        put("gwT" + s, inp["gate_w"][i].T)
        put("gb8" + s, inp["gate_b"][i][:, None])
        put("st8" + s, (SCALE * T2 / inp["temp"][i])[:, None])
        put("svec" + s, pad32_rows(np.repeat(SCALE / inp["temp"][i], DK)).T)
        woT_ = inp["wo"][i].T                               # [dD=16h+d, o2]
        wop = np.zeros((128, 256), np.float32)
        for h in range(H):
            g, j = divmod(h, 4)
            wop[32 * j:32 * j + DK, 128 * g:128 * (g + 1)] = \
                woT_[DK * h:DK * (h + 1), :]
        put("woT" + s, wop)
        put("bo2" + s, (inp["bo"][i] + inp["wo"][i] @ inp["bv"][i])[:, None])
        tu = inp["tu_w"][i]
        mtu = tu.max(1)
        put("tuexpT" + s, np.exp((tu - mtu[:, None]) / T1).T)
        ctu = mtu + inp["tu_b"][i]
        put("ctu" + s, np.stack([ctu[:128], ctu[128:]], 1))
        lfa = np.zeros((128, 16), np.float32)
        lfc = np.zeros((128, 16), np.float32)
        for t in range(2):
            for p in range(P):
                lfa[:, t * 8 + p] = inp["lf_a"][i][p, 128 * t:128 * (t + 1)]
                lfc[:, t * 8 + p] = inp["lf_c"][i][p, 128 * t:128 * (t + 1)]
        put("lfa" + s, lfa)
        put("lfc" + s, lfc)
        gl = 1.0 / (1.0 + np.exp(-inp["lf_gate"][i]))
        put("glc" + s, np.stack([gl[:128], gl[128:]], 1))
        put("glc1m" + s, np.stack([1 - gl[:128], 1 - gl[128:]], 1))
        put("cuT" + s, inp["cu_w"][i].T)
        put("cub" + s, np.stack([inp["cu_b"][i][:128], inp["cu_b"][i][128:]], 1))
        put("fgT" + s, inp["fg_w"][i].T)
        put("fgb" + s, np.stack([inp["fg_b"][i][:128], inp["fg_b"][i][128:]], 1))
        dnT = inp["dn_w"][i].T                                    # [F, D]
        put("dnT" + s, np.concatenate([dnT[:128], dnT[128:]], 1))
        put("dnb" + s, inp["dn_b"][i][:, None])

    put("fnwb", np.stack([inp["fn_w"], inp["fn_b"]], 1))
    hwT = np.zeros((128, 1024), np.float32)
    hb = np.zeros((128, 8), np.float32)
    hw_pad = np.zeros((1024, D), np.float32)
    hw_pad[:NC] = inp["head_w"]
    hb_pad = np.zeros(1024, np.float32)
    hb_pad[:NC] = inp["head_b"]
    for j in range(8):
        hwT[:, 128 * j:128 * (j + 1)] = hw_pad[128 * j:128 * (j + 1)].T
        hb[:, j] = hb_pad[128 * j:128 * (j + 1)]
    put("hwT", hwT)
    put("hb", hb)
    return W


def _prep_x(inp, b):
    xb = inp["x"][b]
    xp = xb.reshape(3, 14, PS, 14, PS).transpose(1, 3, 0, 2, 4).reshape(
        NPATCH, 3 * PS * PS)
    xpT = np.ascontiguousarray(xp.T.astype(np.float32))
    return np.concatenate([xpT[128 * k:128 * (k + 1)] for k in range(KCH)], 1)


# ---------------------------------------------------------------- bass program
def _build_program():
    nc = bacc.Bacc("TRN2", target_bir_lowering=False, debug=False,
                   enable_asserts=True, num_devices=8)
    inp_d = nc.dram_tensor("inp", [128, NCOL], FP, kind="ExternalInput").ap()
    out_d = nc.dram_tensor("out", [128, 8], FP, kind="ExternalOutput").ap()
    with tile.TileContext(nc) as tc:
        _bass_body(nc, tc, inp_d, out_d)
    nc.compile()
    return nc


def _bass_body(nc, tc, inp_d, out_d):
    import contextlib
    ctx = contextlib.ExitStack()
    perm = ctx.enter_context(tc.tile_pool(name="perm", bufs=1))
    work = ctx.enter_context(tc.tile_pool(name="work", bufs=2))
    psp = ctx.enter_context(tc.tile_pool(name="psp", bufs=1, space="PSUM"))

    WPK = perm.tile([128, NCOL], FP)
    nc.sync.dma_start(WPK[:], inp_d[:])

    def wp(name, rows=128):
        o, n = PK.sl(name)
        return WPK[0:rows, o:o + n]

    def col(name, j=0, rows=128):
        o, n = PK.sl(name)
        return WPK[0:rows, o + j:o + j + 1]

    _pp_ctr = [0]

    def pp(shape, tag="pp", bufs=3):
        _pp_ctr[0] += 1
        return psp.tile(shape, FP, tag=tag, bufs=bufs,
                        name=f"{tag}{_pp_ctr[0]}",
                        padded_shape=[128, 512])

    onesrow = wp("onesrow")[0:1, :]
    meancol = wp("meancol")

    hT = perm.tile([128, 2 * L], FP)     # residual cols 0:197, x^2 scratch 197:394

    # ---- patch embed ----
    pe = pp([128, NPATCH])
    xo, _ = PK.sl("xp")
    po, _ = PK.sl("pw")
    for k in range(KCH):
        nc.tensor.matmul(pe[:], WPK[:, po + 128 * k: po + 128 * (k + 1)],
                         WPK[:, xo + NPATCH * k: xo + NPATCH * (k + 1)],
                         start=(k == 0), stop=(k == KCH - 1))
    nc.vector.tensor_tensor(hT[:, 1:L], pe[:], wp("b0")[:, 1:L], op=OP.add)
    nc.vector.tensor_copy(hT[:, 0:1], wp("b0")[:, 0:1])

    def layer_norm(wb_ap):
        sq = hT[:, L:2 * L]
        nc.vector.tensor_tensor(sq, hT[:, 0:L], hT[:, 0:L], op=OP.mult)
        stats = pp([1, 2 * L])
        nc.tensor.matmul(stats[:], meancol, hT[:], start=True, stop=True)
        mean = work.tile([1, L], FP, tag="mean")
        nc.vector.tensor_copy(mean[:], stats[0:1, 0:L])
        msq = work.tile([1, L], FP, tag="msq")
        nc.vector.tensor_tensor(msq[:], mean[:], mean[:], op=OP.mult)
        var = work.tile([1, L], FP, tag="var")
        nc.vector.tensor_tensor(var[:], stats[0:1, L:2 * L], msq[:],
                                op=OP.subtract)
        std = work.tile([1, L], FP, tag="std")
        nc.scalar.activation(std[:], var[:], ACTF.Sqrt, bias=col("epscol", rows=1))
        rstd = work.tile([1, L], FP, tag="rstd")
        nc.vector.reciprocal_approx_fast(out=rstd[:], in_=std[:])
        mr = work.tile([1, L], FP, tag="mr")
        nc.vector.tensor_tensor(mr[:], mean[:], rstd[:], op=OP.mult)
        rstdB = pp([128, L])
        nc.tensor.matmul(rstdB[:], onesrow, rstd[:], start=True, stop=True)
        mrB = pp([128, L])
        nc.tensor.matmul(mrB[:], onesrow, mr[:], start=True, stop=True)
        t1_ = work.tile([128, L], FP, tag="lnt1")
        nc.vector.tensor_tensor(t1_[:], hT[:, 0:L], rstdB[:], op=OP.mult)
        t2_ = work.tile([128, L], FP, tag="lnt2")
        nc.vector.tensor_tensor(t2_[:], t1_[:], mrB[:], op=OP.subtract)
        hn = work.tile([128, L], FP, tag="hn")
        nc.vector.tensor_scalar(hn[:], t2_[:], wb_ap[:, 0:1], wb_ap[:, 1:2],
                                op0=OP.mult, op1=OP.add)
        return hn

    def trop_exp_rhs(hn, mxB):
        from concourse import bass_isa
        nc.gpsimd.partition_all_reduce(mxB[:], hn[:], channels=128,
                                       reduce_op=bass_isa.ReduceOp.max)
        xc = work.tile([128, L], FP, tag="xc")
        nc.vector.tensor_tensor(xc[:], hn[:], mxB[:], op=OP.subtract)
        xe = work.tile([128, L], FP, tag="xe")
        nc.scalar.activation(xe[:], xc[:], ACTF.Exp, scale=1.0 / T1, bias=col("zerocol"))
        return xe

    def trop_project(xe, mxB, lhsT, cvec, out_tile, M=128):
        sp = pp([128, L])
        nc.tensor.matmul(sp[0:M, :], lhsT, xe[:], start=True, stop=True)
        lg = work.tile([128, L], FP, tag="trop_lg")
        nc.scalar.activation(lg[0:M, :], sp[0:M, :], ACTF.Ln, bias=col("clampcol", rows=M))
        nc.vector.tensor_scalar(out_tile[0:M, :], lg[0:M, :], T1, cvec,
                                op0=OP.mult, op1=OP.add)
        nc.vector.tensor_tensor(out_tile[0:M, :], out_tile[0:M, :],
                                mxB[0:M, :], op=OP.add)

    def global_max_exp(src, tag):
        fm = work.tile([128, 1], FP, tag=tag + "fm")
        nc.vector.tensor_reduce(fm[:], src[:], axis=AX.X, op=OP.max)
        gm = work.tile([1, 1], FP, tag=tag + "gm")
        nc.gpsimd.tensor_reduce(gm[:], fm[:], axis=AX.C, op=OP.max)
        gmB = pp([128, 1])
        nc.tensor.matmul(gmB[:], onesrow, gm[:], start=True, stop=True)
        nb = work.tile([128, 1], FP, tag=tag + "nb")
        nc.vector.tensor_scalar(nb[:], gmB[:], -1.0 / T2, None, op0=OP.mult)
        ex = work.tile([128, L], FP, tag=tag + "ex")
        nc.scalar.activation(ex[:], src[:], ACTF.Exp, bias=nb[:], scale=1.0 / T2)
        return ex

    for i in range(NL):
        s = f"_{i}"
        hn = layer_norm(wp("ln1wb" + s))
        mxB = work.tile([128, L], FP, tag="mxB", name=f"mxA{i}")
        xe = trop_exp_rhs(hn, mxB)

        # Q/K tropical projections into 32-padded head layout: [128, 2L],
        # block g holds heads 4g..4g+3 at partition groups 32j (+16 zero rows)
        Qt = work.tile([128, 2 * L], FP, tag="Qt")
        Kt = work.tile([128, 2 * L], FP, tag="Kt")
        for g in range(2):
            trop_project(xe, mxB, wp("wqexpT" + s)[:, 128 * g:128 * (g + 1)],
                         col("cq" + s, j=g), Qt[:, L * g:L * (g + 1)])
            trop_project(xe, mxB, wp("wkexpT" + s)[:, 128 * g:128 * (g + 1)],
                         col("ck" + s, j=g), Kt[:, L * g:L * (g + 1)])

        # V in the same padded layout: Vsb[0:kn, 256t+128g+32j : +16] = V head
        Vsb = work.tile([128, 512], FP, tag="Vsb")
        for t, (k0, kn) in enumerate(((0, KT0), (KT0, KT1))):
            for g in range(2):
                vp = pp([128, 128])
                nc.tensor.matmul(vp[0:kn, :], hn[:, k0:k0 + kn],
                                 wp("wvT" + s)[:, 128 * g:128 * (g + 1)],
                                 start=True, stop=True)
                nc.vector.tensor_copy(
                    Vsb[0:kn, 256 * t + 128 * g:256 * t + 128 * (g + 1)],
                    vp[0:kn, :])

        gp = pp([8, L])
        nc.tensor.matmul(gp[:], wp("gwT" + s), hn[:], start=True, stop=True)
        gsig = work.tile([8, L], FP, tag="gsig")
        nc.scalar.activation(gsig[:], gp[:], ACTF.Sigmoid,
                             bias=col("gb8" + s, rows=8))
        gts = work.tile([8, L], FP, tag="gts")
        nc.vector.tensor_scalar(gts[:], gsig[:], col("st8" + s, rows=8), None,
                                op0=OP.mult)

        # Qcs = (1 - g_seg) * Qt * svec  (svec zero on pad rows)
        gsegB = pp([128, 2 * L])
        for g in range(2):
            nc.tensor.matmul(gsegB[:, L * g:L * (g + 1)],
                             wp("segg", rows=8)[:, 128 * g:128 * (g + 1)],
                             gsig[:], start=True, stop=True)
        qts = work.tile([128, 2 * L], FP, tag="qts")
        nc.vector.tensor_tensor(qts[:], Qt[:], gsegB[:], op=OP.mult)
        qd = work.tile([128, 2 * L], FP, tag="qd")
        nc.vector.tensor_tensor(qd[:], Qt[:], qts[:], op=OP.subtract)
        Qcs = work.tile([128, 2 * L], FP, tag="Qcs")
        for g in range(2):
            nc.vector.tensor_scalar(Qcs[:, L * g:L * (g + 1)],
                                    qd[:, L * g:L * (g + 1)],
                                    col("svec" + s, j=g), None, op0=OP.mult)

        # exp((Qt - gmax)/T2) with pad rows forced to ~0 via padneg bias
        def gmax_exp(srcT, tag):
            from concourse import bass_isa
            fm = work.tile([128, 1], FP, tag=tag + "fm")
            nc.vector.tensor_reduce(fm[:], srcT[:], axis=AX.X, op=OP.max)
            gm = work.tile([128, 1], FP, tag=tag + "gm")
            nc.gpsimd.partition_all_reduce(gm[:], fm[:], channels=128,
                                           reduce_op=bass_isa.ReduceOp.max)
            nb = work.tile([128, 1], FP, tag=tag + "nb")
            nc.vector.tensor_scalar(nb[:], gm[:], -1.0 / T2, col("padneg"),
                                    op0=OP.mult, op1=OP.add)
            ex = work.tile([128, 2 * L], FP, tag=tag + "ex")
            nc.scalar.activation(ex[:], srcT[:], ACTF.Exp, bias=nb[:],
                                 scale=1.0 / T2)
            return ex

        Qe2 = gmax_exp(Qt, "q2")
        Ke2 = gmax_exp(Kt, "k2")

        sig8 = psp.tile([8, L], FP, tag="sig8", padded_shape=[128, 512])
        eo, _ = PK.sl("e8")
        oSums = []
        for g in range(2):
            oA = psp.tile([128, L], FP, tag="oA", padded_shape=[128, 512], name=f"oA{i}{g}")
            oB = psp.tile([128, L], FP, tag="oB", padded_shape=[128, 512], name=f"oB{i}{g}")
            for j in range(4):
                h = 4 * g + j
                ps32 = slice(32 * j, 32 * (j + 1))
                gB = pp([128, L])
                nc.tensor.matmul(gB[:], wp("oneh", rows=8)[:, 128 * h:128 * (h + 1)],
                                 gts[:], start=True, stop=True)
                for t, (k0, kn) in enumerate(((0, KT0), (KT0, KT1))):
                    sts = pp([128, L])
                    nc.tensor.matmul(sts[0:kn, :],
                                     Ke2[ps32, L * g + k0:L * g + k0 + kn],
                                     Qe2[ps32, L * g:L * (g + 1)],
                                     start=True, stop=True,
                                     tile_position=(32 * j, 0))
                    scs = pp([128, L])
                    nc.tensor.matmul(scs[0:kn, :],
                                     Kt[ps32, L * g + k0:L * g + k0 + kn],
                                     Qcs[ps32, L * g:L * (g + 1)],
                                     start=True, stop=True,
                                     tile_position=(32 * j, 0))
                    lg = work.tile([128, L], FP, tag="sc_lg")
                    nc.scalar.activation(lg[0:kn, :], sts[0:kn, :], ACTF.Ln,
                                         bias=col("clampcol", rows=kn))
                    u = work.tile([128, L], FP, tag="sc_u")
                    nc.vector.tensor_tensor(u[0:kn, :], lg[0:kn, :],
                                            gB[0:kn, :], op=OP.mult)
                    u2 = work.tile([128, L], FP, tag="sc_u2")
                    nc.vector.tensor_tensor(u2[0:kn, :], u[0:kn, :],
                                            scs[0:kn, :], op=OP.add)
                    Pt = work.tile([128, L], FP, tag="sc_P")
                    nc.scalar.activation(Pt[0:kn, :], u2[0:kn, :], ACTF.Exp,
                                         bias=col("zerocol", rows=kn))
                    first = (h == 0 and t == 0)
                    last = (h == H - 1 and t == 1)
                    nc.tensor.matmul(
                        sig8[:], WPK[0:kn, eo + 8 * h:eo + 8 * h + 8],
                        Pt[0:kn, :], start=first, stop=last)
                    ot = oA if t == 0 else oB
                    nc.tensor.matmul(
                        ot[ps32, :],
                        Vsb[0:kn, 256 * t + 128 * g + 32 * j:
                            256 * t + 128 * g + 32 * (j + 1)],
                        Pt[0:kn, :], start=True, stop=True,
                        tile_position=(0, 32 * j))
            oSum = work.tile([128, L], FP, tag="oSum", name=f"oSum{i}{g}")
            nc.vector.tensor_copy(oSum[:], oA[:])
            nc.vector.tensor_tensor(oSum[:], oSum[:], oB[:], op=OP.add)
            oSums.append(oSum)
        rs8 = work.tile([8, L], FP, tag="rs8")
        nc.vector.reciprocal_approx_fast(out=rs8[:], in_=sig8[:])
        pj = pp([128, L])
        for g in range(2):
            rsB = pp([128, L])
            nc.tensor.matmul(rsB[:], wp("segg", rows=8)[:, 128 * g:128 * (g + 1)],
                             rs8[:], start=True, stop=True)
            onrm = work.tile([128, L], FP, tag="onrm")
            nc.vector.tensor_tensor(onrm[:], oSums[g][:], rsB[:], op=OP.mult)
            nc.tensor.matmul(pj[:], wp("woT" + s)[:, 128 * g:128 * (g + 1)],
                             onrm[:], start=(g == 0), stop=(g == 1))
        nc.vector.scalar_tensor_tensor(hT[:, 0:L], pj[:], col("bo2" + s),
                                       hT[:, 0:L], op0=OP.add, op1=OP.add)

        # ---- FFN ----
        hn2 = layer_norm(wp("ln2wb" + s))
        mxB2 = work.tile([128, L], FP, tag="mxB", name=f"mxF{i}")
        xe2 = trop_exp_rhs(hn2, mxB2)
        dp = psp.tile([128, L], FP, tag="dp", padded_shape=[128, 512])
        for t in range(2):
            zT = work.tile([128, L], FP, tag="zT")
            trop_project(xe2, mxB2, wp("tuexpT" + s)[:, 128 * t:128 * (t + 1)],
                         col("ctu" + s, j=t), zT)
            zmx = work.tile([128, L], FP, tag="zmx")
            zmn = work.tile([128, L], FP, tag="zmn")
            nc.vector.tensor_scalar(zmx[:], zT[:], col("lfa" + s, j=t * 8),
                                    col("lfc" + s, j=t * 8), op0=OP.mult,
                                    op1=OP.add)
            nc.vector.tensor_copy(zmn[:], zmx[:])
            for p in range(1, P):
                nc.vector._custom_dve(AFF_MAX, out=zmx[:], in0=zT[:],
                                      in1=zmx[:],
                                      s0=col("lfa" + s, j=t * 8 + p),
                                      s1=col("lfc" + s, j=t * 8 + p))
                nc.vector._custom_dve(AFF_MIN, out=zmn[:], in0=zT[:],
                                      in1=zmn[:],
                                      s0=col("lfa" + s, j=t * 8 + p),
                                      s1=col("lfc" + s, j=t * 8 + p))
            trop_t = work.tile([128, L], FP, tag="trop_t")
            nc.vector.tensor_scalar(trop_t[:], zmx[:], col("glc" + s, j=t),
                                    None, op0=OP.mult)
            nc.vector.scalar_tensor_tensor(trop_t[:], zmn[:],
                                           col("glc1m" + s, j=t), trop_t[:],
                                           op0=OP.mult, op1=OP.add)
            cp = pp([128, L])
            nc.tensor.matmul(cp[:], wp("cuT" + s)[:, 128 * t:128 * (t + 1)],
                             hn2[:], start=True, stop=True)
            cls_t = work.tile([128, L], FP, tag="cls_t")
            nc.scalar.activation(cls_t[:], cp[:], ACTF.Gelu,
                                 bias=col("cub" + s, j=t))
            fgp = pp([128, L])
            nc.tensor.matmul(fgp[:], wp("fgT" + s)[:, 128 * t:128 * (t + 1)],
                             hn2[:], start=True, stop=True)
            gf = work.tile([128, L], FP, tag="gf")
            nc.scalar.activation(gf[:], fgp[:], ACTF.Sigmoid,
                                 bias=col("fgb" + s, j=t))
            dt_ = work.tile([128, L], FP, tag="dt_")
            nc.vector.tensor_tensor(dt_[:], trop_t[:], cls_t[:], op=OP.subtract)
            fused = work.tile([128, L], FP, tag="fused")
            nc.vector.tensor_tensor(fused[:], gf[:], dt_[:], op=OP.mult)
            nc.vector.tensor_tensor(fused[:], fused[:], cls_t[:], op=OP.add)
            nc.tensor.matmul(dp[:], wp("dnT" + s)[:, 128 * t:128 * (t + 1)],
                             fused[:], start=(t == 0), stop=(t == 1))
        nc.vector.scalar_tensor_tensor(hT[:, 0:L], dp[:], col("dnb" + s),
                                       hT[:, 0:L], op0=OP.add, op1=OP.add)

    # ---- final LN (cls column only) + head ----
    h0 = work.tile([128, 1], FP, tag="h0")
    nc.vector.tensor_copy(h0[:], hT[:, 0:1])
    sq0 = work.tile([128, 1], FP, tag="sq0")
    nc.vector.tensor_tensor(sq0[:], h0[:], h0[:], op=OP.mult)
    st0 = pp([1, 2])
    nc.tensor.matmul(st0[0:1, 0:1], meancol, h0[:], start=True, stop=True)
    nc.tensor.matmul(st0[0:1, 1:2], meancol, sq0[:], start=True, stop=True)
    mean0 = work.tile([1, 2], FP, tag="mean0")
    nc.vector.tensor_copy(mean0[:], st0[0:1, 0:2])
    var0 = work.tile([1, 1], FP, tag="var0")
    nc.vector.tensor_tensor(var0[:], mean0[0:1, 0:1], mean0[0:1, 0:1],
                            op=OP.mult)
    nc.vector.tensor_tensor(var0[:], mean0[0:1, 1:2], var0[:], op=OP.subtract)
    std0 = work.tile([1, 1], FP, tag="std0")
    nc.scalar.activation(std0[:], var0[:], ACTF.Sqrt, bias=col("epscol", rows=1))
    rstd0 = work.tile([1, 1], FP, tag="rstd0")
    nc.vector.reciprocal_approx_fast(out=rstd0[:], in_=std0[:])
    mrow = work.tile([1, 2], FP, tag="mrow")
    nc.vector.tensor_tensor(mrow[0:1, 0:1], mean0[0:1, 0:1], rstd0[:],
                            op=OP.mult)
    nc.vector.tensor_copy(mrow[0:1, 1:2], rstd0[:])
    mB = pp([128, 2])
    nc.tensor.matmul(mB[:], onesrow, mrow[:], start=True, stop=True)
    t0 = work.tile([128, 1], FP, tag="t0")
    nc.vector.tensor_tensor(t0[:], h0[:], mB[:, 1:2], op=OP.mult)
    nc.vector.tensor_tensor(t0[:], t0[:], mB[:, 0:1], op=OP.subtract)
    hf = work.tile([128, 1], FP, tag="hf")
    nc.vector.tensor_scalar(hf[:], t0[:], wp("fnwb")[:, 0:1],
                            wp("fnwb")[:, 1:2], op0=OP.mult, op1=OP.add)
    hd = pp([128, 8])
    for j in range(8):
        nc.tensor.matmul(hd[:, j:j + 1], wp("hwT")[:, 128 * j:128 * (j + 1)],
                         hf[:], start=True, stop=True)
    ob = work.tile([128, 8], FP, tag="ob")
    nc.vector.tensor_tensor(ob[:], hd[:], wp("hb"), op=OP.add)
    nc.sync.dma_start(out_d[:], ob[:])
    ctx.close()


# ---------------------------------------------------------------- entry point
_NC_CACHE = []


def _get_nc():
    if not _NC_CACHE:
        _NC_CACHE.append(_build_program())
    return _NC_CACHE[0]


def build_in_maps(inputs):
    inputs = {k: np.asarray(v, np.float32) for k, v in inputs.items()}
    Wsh = _prep_pack(inputs)
    B = inputs["x"].shape[0]
    in_maps = []
    o, n = PK.sl("xp")
    for c in range(8):
        Wc = Wsh.copy()
        Wc[:, o:o + n] = _prep_x(inputs, c % B)
        in_maps.append({"inp": Wc})
    return in_maps


def kernel(**inputs):
    nc = _get_nc()
    in_maps = build_in_maps(inputs)
    B = np.asarray(inputs["x"]).shape[0]
    res = bass_utils.run_bass_kernel_spmd(nc, in_maps, core_ids=list(range(8)))
    outs = []
    for b in range(B):
        om = res.results[b]["out"]
        outs.append(om.T.reshape(-1)[:NC])
    return np.stack(outs).astype(np.float32)


# revision 14
# speedup vs baseline: 173.1312x; 173.1312x over previous
"""Trainium2 Bass kernel for nn_DeepTropNet (dense tropical transformer).

Strategy:
- Batch-parallel across cores (B=4 -> cores 0..3; cores 4..7 duplicate).
- Residual stream kept TRANSPOSED in SBUF: hT [D=128 partitions, L=197 free],
  so every projection is a natural TensorE matmul (contraction on partitions).
- All tropical (max-plus) contractions use the log-sum-exp trick at low
  temperature: max_i(a_i+b_i) = T*log(sum_i e^{a_i/T} e^{b_i/T}), separable ->
  a TensorE matmul of elementwise exponentials. Stabilizers are data-derived
  maxima (host-side for weights, on-device for activations).
- All weight transposes/exp-tables/bias-folds are host-side numpy prep.
- Custom DVE ops (affine_then_max/min) fuse the piecewise-linear fold.
"""
import sys

sys.path.insert(0, "/opt/trn_rl_repo")

import numpy as np

import concourse.bass as bass
import concourse.tile as tile
from concourse import bacc, mybir
from concourse import bass_utils

FP = mybir.dt.float32
AX = mybir.AxisListType
OP = mybir.AluOpType
ACTF = mybir.ActivationFunctionType

NL, D, H, DK, L, F, P, NC, PS = 2, 128, 8, 16, 197, 256, 8, 1000, 16
EPS = 1e-5
SCALE = DK ** -0.5
T1 = 0.01        # temperature for D=128-contraction tropical matmuls (Q,K,z)
T2 = 0.06        # temperature for DK=16-contraction tropical scores
NPATCH = 196
KCH = 6          # 768/128 contraction chunks for patch embed
KT0, KT1 = 128, L - 128   # score k-tiles


# ---------------------------------------------------------------- custom DVE ops
def _make_op(name, body_fn, reference):
    from concourse.dve_spec import Spec, lower, _has_src1
    from concourse.dve_uop import DveOpSpec
    import concourse.dve_ops as dve_ops

    for o in dve_ops.OPS:
        if o.name == name:
            return o
    spec = Spec(body=body_fn(), reference=reference)
    row = dve_ops._CUSTOM_DVE_ROW_BASE + len(dve_ops.OPS)
    assert row < 0x20
    dve_ops._SUB_OPCODE_FOR_NAME[name] = row
    shas = {}
    for ver in ("v3", "v4"):
        try:
            uops = lower(spec, ver=ver)
            shas[ver] = DveOpSpec(name=name, opcode=row, uops=uops,
                                  rd1_en=_has_src1(spec)).sha(ver)
        except Exception:
            pass
    op = dve_ops.DveOp(name, spec, subdim=False, uops_sha=shas)
    dve_ops.OPS.append(op)
    dve_ops.CUSTOM_DVE_SPECS[name] = spec
    return op


def _register_ops():
    from concourse.dve_spec import C0, C1, Src0, Src1, maxx, minn

    aff_max = _make_op(
        "ANT_AFFINE_THEN_MAX",
        lambda: maxx(Src0 * C0 + C1, Src1),
        lambda in0, in1, s0, s1, imm2: np.maximum(
            (in0.astype(np.float32) * s0 + s1), in1).astype(np.float32),
    )
    aff_min = _make_op(
        "ANT_AFFINE_THEN_MIN",
        lambda: minn(Src0 * C0 + C1, Src1),
        lambda in0, in1, s0, s1, imm2: np.minimum(
            (in0.astype(np.float32) * s0 + s1), in1).astype(np.float32),
    )
    return aff_max, aff_min


AFF_MAX, AFF_MIN = _register_ops()


# ---------------------------------------------------------------- host-side prep
class _Pack:
    def __init__(self):
        self.cols = {}
        self.n = 0

    def add(self, name, ncols):
        self.cols[name] = (self.n, ncols)
        self.n += ncols

    def sl(self, name):
        return self.cols[name]


def _layout():
    pk = _Pack()
    pk.add("xp", KCH * NPATCH)        # per-batch patch data chunks
    pk.add("b0", L)
    pk.add("pw", KCH * 128)
    pk.add("onescol", 1)
    pk.add("meancol", 1)              # 1/128
    pk.add("e8", 8 * 8)               # unit-column blocks for sigma matmuls
    pk.add("segg", 2 * 128)           # per-group head segment lhsT (rows 0:8)
    pk.add("oneh", 8 * 128)           # row-h all-ones lhsT blocks (rows 0:8)
    pk.add("onesrow", 128)            # all ones; row 0 used as [1,128] lhsT
    pk.add("epscol", 1)
    pk.add("clampcol", 1)
    pk.add("zerocol", 1)
    pk.add("padneg", 1)
    for i in range(NL):
        s = f"_{i}"
        pk.add("ln1wb" + s, 2)
        pk.add("ln2wb" + s, 2)
        pk.add("wqexpT" + s, 256)
        pk.add("cq" + s, 2)
        pk.add("wkexpT" + s, 256)
        pk.add("ck" + s, 2)
        pk.add("wvT" + s, 256)
        pk.add("gwT" + s, 8)
        pk.add("gb8" + s, 1)          # rows 0:8
        pk.add("st8" + s, 1)          # rows 0:8: scale*T2/temp_h
        pk.add("svec" + s, 2)         # per-group scale/temp cols (pad rows 0)
        pk.add("woT" + s, 256)
        pk.add("bo2" + s, 1)
        pk.add("tuexpT" + s, F)
        pk.add("ctu" + s, 2)
        pk.add("lfa" + s, 16)         # col t*8+p
        pk.add("lfc" + s, 16)
        pk.add("glc" + s, 2)
        pk.add("glc1m" + s, 2)
        pk.add("cuT" + s, F)
        pk.add("cub" + s, 2)
        pk.add("fgT" + s, F)
        pk.add("fgb" + s, 2)
        pk.add("dnT" + s, 2 * 128)
        pk.add("dnb" + s, 1)
    pk.add("fnwb", 2)
    pk.add("hwT", 1024)
    pk.add("hb", 8)
    return pk


PK = _layout()
NCOL = PK.n


def _prep_pack(inp):
    W = np.zeros((128, NCOL), np.float32)

    def put(name, arr):
        o, n = PK.sl(name)
        a = np.asarray(arr, np.float32)
        assert a.ndim == 2 and a.shape[1] == n, (name, a.shape, n)
        W[: a.shape[0], o:o + n] = a

    bn_s = inp["bn_gamma"] / (inp["bn_run_range"] + EPS)
    bn_b = inp["bn_beta"] - inp["bn_run_max"] * bn_s
    pos = inp["pos_embed"][0]                                     # [L, D]
    clsb = inp["cls_token"].reshape(D)
    b0 = np.empty((D, L), np.float32)
    b0[:, 0] = bn_s * (clsb + pos[0]) + bn_b
    b0[:, 1:] = (bn_s[:, None] * (inp["patch_b"][:, None] + pos[1:].T)
                 + bn_b[:, None])
    put("b0", b0)

    pwT = (bn_s[:, None] * inp["patch_w"]).T                      # [768, D]
    put("pw", np.concatenate([pwT[128 * k:128 * (k + 1)] for k in range(KCH)], 1))

    put("onescol", np.ones((128, 1)))
    put("meancol", np.full((128, 1), 1.0 / D))
    e8 = np.zeros((128, 64), np.float32)
    for h in range(H):
        e8[:, 8 * h + h] = 1.0
    put("e8", e8)
    segg = np.zeros((8, 2 * 128), np.float32)
    for g in range(2):
        for j in range(4):
            segg[4 * g + j, 128 * g + 32 * j:128 * g + 32 * j + DK] = 1.0
    put("segg", segg)
    oneh = np.zeros((8, 8 * 128), np.float32)
    for h in range(H):
        oneh[h, 128 * h:128 * (h + 1)] = 1.0
    put("oneh", oneh)
    put("onesrow", np.ones((128, 128)))
    put("epscol", np.full((128, 1), EPS))
    put("clampcol", np.full((128, 1), 1e-30))
    put("zerocol", np.zeros((128, 1)))
    padneg = np.zeros((128, 1), np.float32)
    for j in range(4):
        padneg[32 * j + DK:32 * (j + 1)] = -1e5
    put("padneg", padneg)

    def pad32_rows(vec):
        # [D] head-indexed (16h+d) -> two [128] group columns at rows 32j+d
        out = np.zeros((2, 128), np.float32)
        for h in range(H):
            g, j = divmod(h, 4)
            out[g, 32 * j:32 * j + DK] = vec[DK * h:DK * (h + 1)]
        return out

    for i in range(NL):
        s = f"_{i}"
        put("ln1wb" + s, np.stack([inp["n1_w"][i], inp["n1_b"][i]], 1))
        put("ln2wb" + s, np.stack([inp["n2_w"][i], inp["n2_b"][i]], 1))
        for nm, bkey in (("wq", "bq"), ("wk", "bk")):
            Wt = inp[nm][i]
            mW = Wt.max(1)
            we = np.exp((Wt - mW[:, None]) / T1).T          # [i, o=16h+d]
            wep = np.zeros((128, 256), np.float32)
            for h in range(H):
                g, j = divmod(h, 4)
                wep[:, 128 * g + 32 * j:128 * g + 32 * j + DK] = \
                    we[:, DK * h:DK * (h + 1)]
            put(nm + "expT" + s, wep)
            put("c" + nm[1] + s, pad32_rows(mW + inp[bkey][i]).T)
        wvp = np.zeros((128, 256), np.float32)
        wvT_ = inp["wv"][i].T                               # [i, 16h+d]
        for h in range(H):
            g, j = divmod(h, 4)
            wvp[:, 128 * g + 32 * j:128 * g + 32 * j + DK] = \
                wvT_[:, DK * h:DK * (h + 1)]
        put("wvT" + s, wvp)
        put("gwT" + s, inp["gate_w"][i].T)
        put("gb8" + s, inp["gate_b"][i][:, None])
        put("st8" + s, (SCALE * T2 / inp["temp"][i])[:, None])
        put("svec" + s, pad32_rows(np.repeat(SCALE / inp["temp"][i], DK)).T)
        woT_ = inp["wo"][i].T                               # [dD=16h+d, o2]
        wop = np.zeros((128, 256), np.float32)
        for h in range(H):
            g, j = divmod(h, 4)
            wop[32 * j:32 * j + DK, 128 * g:128 * (g + 1)] = \
                woT_[DK * h:DK * (h + 1), :]
        put("woT" + s, wop)
        put("bo2" + s, (inp["bo"][i] + inp["wo"][i] @ inp["bv"][i])[:, None])
        tu = inp["tu_w"][i]
        mtu = tu.max(1)
        put("tuexpT" + s, np.exp((tu - mtu[:, None]) / T1).T)
        ctu = mtu + inp["tu_b"][i]
        put("ctu" + s, np.stack([ctu[:128], ctu[128:]], 1))
        lfa = np.zeros((128, 16), np.float32)
        lfc = np.zeros((128, 16), np.float32)
        for t in range(2):
            for p in range(P):
                lfa[:, t * 8 + p] = inp["lf_a"][i][p, 128 * t:128 * (t + 1)]
                lfc[:, t * 8 + p] = inp["lf_c"][i][p, 128 * t:128 * (t + 1)]
        put("lfa" + s, lfa)
        put("lfc" + s, lfc)
        gl = 1.0 / (1.0 + np.exp(-inp["lf_gate"][i]))
        put("glc" + s, np.stack([gl[:128], gl[128:]], 1))
        put("glc1m" + s, np.stack([1 - gl[:128], 1 - gl[128:]], 1))
        put("cuT" + s, inp["cu_w"][i].T)
        put("cub" + s, np.stack([inp["cu_b"][i][:128], inp["cu_b"][i][128:]], 1))
        put("fgT" + s, inp["fg_w"][i].T)
        put("fgb" + s, np.stack([inp["fg_b"][i][:128], inp["fg_b"][i][128:]], 1))
        dnT = inp["dn_w"][i].T                                    # [F, D]
        put("dnT" + s, np.concatenate([dnT[:128], dnT[128:]], 1))
        put("dnb" + s, inp["dn_b"][i][:, None])

    put("fnwb", np.stack([inp["fn_w"], inp["fn_b"]], 1))
    hwT = np.zeros((128, 1024), np.float32)
    hb = np.zeros((128, 8), np.float32)
    hw_pad = np.zeros((1024, D), np.float32)
    hw_pad[:NC] = inp["head_w"]
    hb_pad = np.zeros(1024, np.float32)
    hb_pad[:NC] = inp["head_b"]
    for j in range(8):
        hwT[:, 128 * j:128 * (j + 1)] = hw_pad[128 * j:128 * (j + 1)].T
        hb[:, j] = hb_pad[128 * j:128 * (j + 1)]
    put("hwT", hwT)
    put("hb", hb)
    return W


def _prep_x(inp, b):
    xb = inp["x"][b]
    xp = xb.reshape(3, 14, PS, 14, PS).transpose(1, 3, 0, 2, 4).reshape(
        NPATCH, 3 * PS * PS)
    xpT = np.ascontiguousarray(xp.T.astype(np.float32))
    return np.concatenate([xpT[128 * k:128 * (k + 1)] for k in range(KCH)], 1)


# ---------------------------------------------------------------- bass program
def _build_program(reps=1):
    nc = bacc.Bacc("TRN2", target_bir_lowering=False, debug=False,
                   enable_asserts=True, num_devices=8)
    inp_d = nc.dram_tensor("inp", [128, NCOL], FP, kind="ExternalInput").ap()
    out_d = nc.dram_tensor("out", [128, 8], FP, kind="ExternalOutput").ap()
    with tile.TileContext(nc) as tc:
        _bass_body(nc, tc, inp_d, out_d, reps=reps)
    nc.compile()
    return nc


def _bass_body(nc, tc, inp_d, out_d, reps=1):
    import contextlib
    ctx = contextlib.ExitStack()
    perm = ctx.enter_context(tc.tile_pool(name="perm", bufs=1))
    work = ctx.enter_context(tc.tile_pool(name="work", bufs=2))
    psp = ctx.enter_context(tc.tile_pool(name="psp", bufs=1, space="PSUM"))

    WPK = perm.tile([128, NCOL], FP)
    nc.sync.dma_start(WPK[:], inp_d[:])

    def wp(name, rows=128):
        o, n = PK.sl(name)
        return WPK[0:rows, o:o + n]

    def col(name, j=0, rows=128):
        o, n = PK.sl(name)
        return WPK[0:rows, o + j:o + j + 1]

    _pp_ctr = [0]

    def pp(shape, tag="pp", bufs=3):
        _pp_ctr[0] += 1
        return psp.tile(shape, FP, tag=tag, bufs=bufs,
                        name=f"{tag}{_pp_ctr[0]}",
                        padded_shape=[128, 512])

    onesrow = wp("onesrow")[0:1, :]
    meancol = wp("meancol")

    hT = perm.tile([128, 2 * L], FP)     # residual cols 0:197, x^2 scratch 197:394

    if reps > 1:
        loop_cm = tc.For_i(0, reps, 1)
        loop_cm.__enter__()

    # ---- patch embed ----
    pe = pp([128, NPATCH])
    xo, _ = PK.sl("xp")
    po, _ = PK.sl("pw")
    for k in range(KCH):
        nc.tensor.matmul(pe[:], WPK[:, po + 128 * k: po + 128 * (k + 1)],
                         WPK[:, xo + NPATCH * k: xo + NPATCH * (k + 1)],
                         start=(k == 0), stop=(k == KCH - 1))
    nc.vector.tensor_tensor(hT[:, 1:L], pe[:], wp("b0")[:, 1:L], op=OP.add)
    nc.vector.tensor_copy(hT[:, 0:1], wp("b0")[:, 0:1])

    def layer_norm(wb_ap):
        sq = hT[:, L:2 * L]
        nc.vector.tensor_tensor(sq, hT[:, 0:L], hT[:, 0:L], op=OP.mult)
        stats = pp([1, 2 * L])
        nc.tensor.matmul(stats[:], meancol, hT[:], start=True, stop=True)
        mean = work.tile([1, L], FP, tag="mean")
        nc.vector.tensor_copy(mean[:], stats[0:1, 0:L])
        msq = work.tile([1, L], FP, tag="msq")
        nc.vector.tensor_tensor(msq[:], mean[:], mean[:], op=OP.mult)
        var = work.tile([1, L], FP, tag="var")
        nc.vector.tensor_tensor(var[:], stats[0:1, L:2 * L], msq[:],
                                op=OP.subtract)
        std = work.tile([1, L], FP, tag="std")
        nc.scalar.activation(std[:], var[:], ACTF.Sqrt, bias=col("epscol", rows=1))
        rstd = work.tile([1, L], FP, tag="rstd")
        nc.vector.reciprocal_approx_fast(out=rstd[:], in_=std[:])
        mr = work.tile([1, L], FP, tag="mr")
        nc.vector.tensor_tensor(mr[:], mean[:], rstd[:], op=OP.mult)
        rstdB = pp([128, L])
        nc.tensor.matmul(rstdB[:], onesrow, rstd[:], start=True, stop=True)
        mrB = pp([128, L])
        nc.tensor.matmul(mrB[:], onesrow, mr[:], start=True, stop=True)
        t1_ = work.tile([128, L], FP, tag="lnt1")
        nc.vector.tensor_tensor(t1_[:], hT[:, 0:L], rstdB[:], op=OP.mult)
        t2_ = work.tile([128, L], FP, tag="lnt2")
        nc.vector.tensor_tensor(t2_[:], t1_[:], mrB[:], op=OP.subtract)
        hn = work.tile([128, L], FP, tag="hn")
        nc.vector.tensor_scalar(hn[:], t2_[:], wb_ap[:, 0:1], wb_ap[:, 1:2],
                                op0=OP.mult, op1=OP.add)
        return hn

    def trop_exp_rhs(hn, mxB):
        from concourse import bass_isa
        nc.gpsimd.partition_all_reduce(mxB[:], hn[:], channels=128,
                                       reduce_op=bass_isa.ReduceOp.max)
        xc = work.tile([128, L], FP, tag="xc")
        nc.vector.tensor_tensor(xc[:], hn[:], mxB[:], op=OP.subtract)
        xe = work.tile([128, L], FP, tag="xe")
        nc.scalar.activation(xe[:], xc[:], ACTF.Exp, scale=1.0 / T1, bias=col("zerocol"))
        return xe

    def trop_project(xe, mxB, lhsT, cvec, out_tile, M=128):
        sp = pp([128, L])
        nc.tensor.matmul(sp[0:M, :], lhsT, xe[:], start=True, stop=True)
        lg = work.tile([128, L], FP, tag="trop_lg")
        nc.scalar.activation(lg[0:M, :], sp[0:M, :], ACTF.Ln, bias=col("clampcol", rows=M))
        nc.vector.tensor_scalar(out_tile[0:M, :], lg[0:M, :], T1, cvec,
                                op0=OP.mult, op1=OP.add)
        nc.vector.tensor_tensor(out_tile[0:M, :], out_tile[0:M, :],
                                mxB[0:M, :], op=OP.add)

    def global_max_exp(src, tag):
        fm = work.tile([128, 1], FP, tag=tag + "fm")
        nc.vector.tensor_reduce(fm[:], src[:], axis=AX.X, op=OP.max)
        gm = work.tile([1, 1], FP, tag=tag + "gm")
        nc.gpsimd.tensor_reduce(gm[:], fm[:], axis=AX.C, op=OP.max)
        gmB = pp([128, 1])
        nc.tensor.matmul(gmB[:], onesrow, gm[:], start=True, stop=True)
        nb = work.tile([128, 1], FP, tag=tag + "nb")
        nc.vector.tensor_scalar(nb[:], gmB[:], -1.0 / T2, None, op0=OP.mult)
        ex = work.tile([128, L], FP, tag=tag + "ex")
        nc.scalar.activation(ex[:], src[:], ACTF.Exp, bias=nb[:], scale=1.0 / T2)
        return ex

    for i in range(NL):
        s = f"_{i}"
        hn = layer_norm(wp("ln1wb" + s))
        mxB = work.tile([128, L], FP, tag="mxB", name=f"mxA{i}")
        xe = trop_exp_rhs(hn, mxB)

        # Q/K tropical projections into 32-padded head layout: [128, 2L],
        # block g holds heads 4g..4g+3 at partition groups 32j (+16 zero rows)
        Qt = work.tile([128, 2 * L], FP, tag="Qt")
        Kt = work.tile([128, 2 * L], FP, tag="Kt")
        for g in range(2):
            trop_project(xe, mxB, wp("wqexpT" + s)[:, 128 * g:128 * (g + 1)],
                         col("cq" + s, j=g), Qt[:, L * g:L * (g + 1)])
            trop_project(xe, mxB, wp("wkexpT" + s)[:, 128 * g:128 * (g + 1)],
                         col("ck" + s, j=g), Kt[:, L * g:L * (g + 1)])

        # V in the same padded layout: Vsb[0:kn, 256t+128g+32j : +16] = V head
        Vsb = work.tile([128, 512], FP, tag="Vsb")
        for t, (k0, kn) in enumerate(((0, KT0), (KT0, KT1))):
            for g in range(2):
                vp = pp([128, 128])
                nc.tensor.matmul(vp[0:kn, :], hn[:, k0:k0 + kn],
                                 wp("wvT" + s)[:, 128 * g:128 * (g + 1)],
                                 start=True, stop=True)
                nc.vector.tensor_copy(
                    Vsb[0:kn, 256 * t + 128 * g:256 * t + 128 * (g + 1)],
                    vp[0:kn, :])

        gp = pp([8, L])
        nc.tensor.matmul(gp[:], wp("gwT" + s), hn[:], start=True, stop=True)
        gsig = work.tile([8, L], FP, tag="gsig")
        nc.scalar.activation(gsig[:], gp[:], ACTF.Sigmoid,
                             bias=col("gb8" + s, rows=8))
        gts = work.tile([8, L], FP, tag="gts")
        nc.vector.tensor_scalar(gts[:], gsig[:], col("st8" + s, rows=8), None,
                                op0=OP.mult)

        # Qcs = (1 - g_seg) * Qt * svec  (svec zero on pad rows)
        gsegB = pp([128, 2 * L])
        for g in range(2):
            nc.tensor.matmul(gsegB[:, L * g:L * (g + 1)],
                             wp("segg", rows=8)[:, 128 * g:128 * (g + 1)],
                             gsig[:], start=True, stop=True)
        qts = work.tile([128, 2 * L], FP, tag="qts")
        nc.vector.tensor_tensor(qts[:], Qt[:], gsegB[:], op=OP.mult)
        qd = work.tile([128, 2 * L], FP, tag="qd")
        nc.vector.tensor_tensor(qd[:], Qt[:], qts[:], op=OP.subtract)
        Qcs = work.tile([128, 2 * L], FP, tag="Qcs")
        for g in range(2):
            nc.vector.tensor_scalar(Qcs[:, L * g:L * (g + 1)],
                                    qd[:, L * g:L * (g + 1)],
                                    col("svec" + s, j=g), None, op0=OP.mult)

        # exp((Qt - gmax)/T2) with pad rows forced to ~0 via padneg bias
        def gmax_exp(srcT, tag):
            from concourse import bass_isa
            fm = work.tile([128, 1], FP, tag=tag + "fm")
            nc.vector.tensor_reduce(fm[:], srcT[:], axis=AX.X, op=OP.max)
            gm = work.tile([128, 1], FP, tag=tag + "gm")
            nc.gpsimd.partition_all_reduce(gm[:], fm[:], channels=128,
                                           reduce_op=bass_isa.ReduceOp.max)
            nb = work.tile([128, 1], FP, tag=tag + "nb")
            nc.vector.tensor_scalar(nb[:], gm[:], -1.0 / T2, col("padneg"),
                                    op0=OP.mult, op1=OP.add)
            ex = work.tile([128, 2 * L], FP, tag=tag + "ex")
            nc.scalar.activation(ex[:], srcT[:], ACTF.Exp, bias=nb[:],
                                 scale=1.0 / T2)
            return ex

        Qe2 = gmax_exp(Qt, "q2")
        Ke2 = gmax_exp(Kt, "k2")

        sig8 = psp.tile([8, L], FP, tag="sig8", padded_shape=[128, 512])
        eo, _ = PK.sl("e8")
        oSums = []
        for g in range(2):
            oA = psp.tile([128, L], FP, tag="oA", padded_shape=[128, 512], name=f"oA{i}{g}")
            oB = psp.tile([128, L], FP, tag="oB", padded_shape=[128, 512], name=f"oB{i}{g}")
            for j in range(4):
                h = 4 * g + j
                ps32 = slice(32 * j, 32 * (j + 1))
                gB = pp([128, L])
                nc.tensor.matmul(gB[:], wp("oneh", rows=8)[:, 128 * h:128 * (h + 1)],
                                 gts[:], start=True, stop=True)
                for t, (k0, kn) in enumerate(((0, KT0), (KT0, KT1))):
                    sts = pp([128, L])
                    nc.tensor.matmul(sts[0:kn, :],
                                     Ke2[ps32, L * g + k0:L * g + k0 + kn],
                                     Qe2[ps32, L * g:L * (g + 1)],
                                     start=True, stop=True,
                                     tile_position=(32 * j, 0))
                    scs = pp([128, L])
                    nc.tensor.matmul(scs[0:kn, :],
                                     Kt[ps32, L * g + k0:L * g + k0 + kn],
                                     Qcs[ps32, L * g:L * (g + 1)],
                                     start=True, stop=True,
                                     tile_position=(32 * j, 0))
                    lg = work.tile([128, L], FP, tag="sc_lg")
                    nc.scalar.activation(lg[0:kn, :], sts[0:kn, :], ACTF.Ln,
                                         bias=col("clampcol", rows=kn))
                    u = work.tile([128, L], FP, tag="sc_u")
                    nc.vector.tensor_tensor(u[0:kn, :], lg[0:kn, :],
                                            gB[0:kn, :], op=OP.mult)
                    u2 = work.tile([128, L], FP, tag="sc_u2")
                    nc.vector.tensor_tensor(u2[0:kn, :], u[0:kn, :],
                                            scs[0:kn, :], op=OP.add)
                    Pt = work.tile([128, L], FP, tag="sc_P")
                    nc.scalar.activation(Pt[0:kn, :], u2[0:kn, :], ACTF.Exp,
                                         bias=col("zerocol", rows=kn))
                    first = (h == 0 and t == 0)
                    last = (h == H - 1 and t == 1)
                    nc.tensor.matmul(
                        sig8[:], WPK[0:kn, eo + 8 * h:eo + 8 * h + 8],
                        Pt[0:kn, :], start=first, stop=last)
                    ot = oA if t == 0 else oB
                    nc.tensor.matmul(
                        ot[ps32, :],
                        Vsb[0:kn, 256 * t + 128 * g + 32 * j:
                            256 * t + 128 * g + 32 * (j + 1)],
                        Pt[0:kn, :], start=True, stop=True,
                        tile_position=(0, 32 * j))
            oSum = work.tile([128, L], FP, tag="oSum", name=f"oSum{i}{g}")
            nc.vector.tensor_copy(oSum[:], oA[:])
            nc.vector.tensor_tensor(oSum[:], oSum[:], oB[:], op=OP.add)
            oSums.append(oSum)
        rs8 = work.tile([8, L], FP, tag="rs8")
        nc.vector.reciprocal_approx_fast(out=rs8[:], in_=sig8[:])
        pj = pp([128, L])
        for g in range(2):
            rsB = pp([128, L])
            nc.tensor.matmul(rsB[:], wp("segg", rows=8)[:, 128 * g:128 * (g + 1)],
                             rs8[:], start=True, stop=True)
            onrm = work.tile([128, L], FP, tag="onrm")
            nc.vector.tensor_tensor(onrm[:], oSums[g][:], rsB[:], op=OP.mult)
            nc.tensor.matmul(pj[:], wp("woT" + s)[:, 128 * g:128 * (g + 1)],
                             onrm[:], start=(g == 0), stop=(g == 1))
        nc.vector.scalar_tensor_tensor(hT[:, 0:L], pj[:], col("bo2" + s),
                                       hT[:, 0:L], op0=OP.add, op1=OP.add)

        # ---- FFN ----
        hn2 = layer_norm(wp("ln2wb" + s))
        mxB2 = work.tile([128, L], FP, tag="mxB", name=f"mxF{i}")
        xe2 = trop_exp_rhs(hn2, mxB2)
        dp = psp.tile([128, L], FP, tag="dp", padded_shape=[128, 512])
        for t in range(2):
            zT = work.tile([128, L], FP, tag="zT")
            trop_project(xe2, mxB2, wp("tuexpT" + s)[:, 128 * t:128 * (t + 1)],
                         col("ctu" + s, j=t), zT)
            zmx = work.tile([128, L], FP, tag="zmx")
            zmn = work.tile([128, L], FP, tag="zmn")
            nc.vector.tensor_scalar(zmx[:], zT[:], col("lfa" + s, j=t * 8),
                                    col("lfc" + s, j=t * 8), op0=OP.mult,
                                    op1=OP.add)
            nc.vector.tensor_copy(zmn[:], zmx[:])
            for p in range(1, P):
                nc.vector._custom_dve(AFF_MAX, out=zmx[:], in0=zT[:],
                                      in1=zmx[:],
                                      s0=col("lfa" + s, j=t * 8 + p),
                                      s1=col("lfc" + s, j=t * 8 + p))
                nc.vector._custom_dve(AFF_MIN, out=zmn[:], in0=zT[:],
                                      in1=zmn[:],
                                      s0=col("lfa" + s, j=t * 8 + p),
                                      s1=col("lfc" + s, j=t * 8 + p))
            trop_t = work.tile([128, L], FP, tag="trop_t")
            nc.vector.tensor_scalar(trop_t[:], zmx[:], col("glc" + s, j=t),
                                    None, op0=OP.mult)
            nc.vector.scalar_tensor_tensor(trop_t[:], zmn[:],
                                           col("glc1m" + s, j=t), trop_t[:],
                                           op0=OP.mult, op1=OP.add)
            cp = pp([128, L])
            nc.tensor.matmul(cp[:], wp("cuT" + s)[:, 128 * t:128 * (t + 1)],
                             hn2[:], start=True, stop=True)
            cls_t = work.tile([128, L], FP, tag="cls_t")
            nc.scalar.activation(cls_t[:], cp[:], ACTF.Gelu,
                                 bias=col("cub" + s, j=t))
            fgp = pp([128, L])
            nc.tensor.matmul(fgp[:], wp("fgT" + s)[:, 128 * t:128 * (t + 1)],
                             hn2[:], start=True, stop=True)
            gf = work.tile([128, L], FP, tag="gf")
            nc.scalar.activation(gf[:], fgp[:], ACTF.Sigmoid,
                                 bias=col("fgb" + s, j=t))
            dt_ = work.tile([128, L], FP, tag="dt_")
            nc.vector.tensor_tensor(dt_[:], trop_t[:], cls_t[:], op=OP.subtract)
            fused = work.tile([128, L], FP, tag="fused")
            nc.vector.tensor_tensor(fused[:], gf[:], dt_[:], op=OP.mult)
            nc.vector.tensor_tensor(fused[:], fused[:], cls_t[:], op=OP.add)
            nc.tensor.matmul(dp[:], wp("dnT" + s)[:, 128 * t:128 * (t + 1)],
                             fused[:], start=(t == 0), stop=(t == 1))
        nc.vector.scalar_tensor_tensor(hT[:, 0:L], dp[:], col("dnb" + s),
                                       hT[:, 0:L], op0=OP.add, op1=OP.add)

    # ---- final LN (cls column only) + head ----
    h0 = work.tile([128, 1], FP, tag="h0")
    nc.vector.tensor_copy(h0[:], hT[:, 0:1])
    sq0 = work.tile([128, 1], FP, tag="sq0")
    nc.vector.tensor_tensor(sq0[:], h0[:], h0[:], op=OP.mult)
    st0 = pp([1, 2])
    nc.tensor.matmul(st0[0:1, 0:1], meancol, h0[:], start=True, stop=True)
    nc.tensor.matmul(st0[0:1, 1:2], meancol, sq0[:], start=True, stop=True)
    mean0 = work.tile([1, 2], FP, tag="mean0")
    nc.vector.tensor_copy(mean0[:], st0[0:1, 0:2])
    var0 = work.tile([1, 1], FP, tag="var0")
    nc.vector.tensor_tensor(var0[:], mean0[0:1, 0:1], mean0[0:1, 0:1],
                            op=OP.mult)
    nc.vector.tensor_tensor(var0[:], mean0[0:1, 1:2], var0[:], op=OP.subtract)
    std0 = work.tile([1, 1], FP, tag="std0")
    nc.scalar.activation(std0[:], var0[:], ACTF.Sqrt, bias=col("epscol", rows=1))
    rstd0 = work.tile([1, 1], FP, tag="rstd0")
    nc.vector.reciprocal_approx_fast(out=rstd0[:], in_=std0[:])
    mrow = work.tile([1, 2], FP, tag="mrow")
    nc.vector.tensor_tensor(mrow[0:1, 0:1], mean0[0:1, 0:1], rstd0[:],
                            op=OP.mult)
    nc.vector.tensor_copy(mrow[0:1, 1:2], rstd0[:])
    mB = pp([128, 2])
    nc.tensor.matmul(mB[:], onesrow, mrow[:], start=True, stop=True)
    t0 = work.tile([128, 1], FP, tag="t0")
    nc.vector.tensor_tensor(t0[:], h0[:], mB[:, 1:2], op=OP.mult)
    nc.vector.tensor_tensor(t0[:], t0[:], mB[:, 0:1], op=OP.subtract)
    hf = work.tile([128, 1], FP, tag="hf")
    nc.vector.tensor_scalar(hf[:], t0[:], wp("fnwb")[:, 0:1],
                            wp("fnwb")[:, 1:2], op0=OP.mult, op1=OP.add)
    hd = pp([128, 8])
    for j in range(8):
        nc.tensor.matmul(hd[:, j:j + 1], wp("hwT")[:, 128 * j:128 * (j + 1)],
                         hf[:], start=True, stop=True)
    ob = work.tile([128, 8], FP, tag="ob")
    nc.vector.tensor_tensor(ob[:], hd[:], wp("hb"), op=OP.add)
    if reps > 1:
        loop_cm.__exit__(None, None, None)
    nc.sync.dma_start(out_d[:], ob[:])
    ctx.close()


# ---------------------------------------------------------------- entry point
_NC_CACHE = []


def _get_nc():
    if not _NC_CACHE:
        _NC_CACHE.append(_build_program())
    return _NC_CACHE[0]


def build_in_maps(inputs):
    inputs = {k: np.asarray(v, np.float32) for k, v in inputs.items()}
    Wsh = _prep_pack(inputs)
    B = inputs["x"].shape[0]
    in_maps = []
    o, n = PK.sl("xp")
    for c in range(8):
        Wc = Wsh.copy()
        Wc[:, o:o + n] = _prep_x(inputs, c % B)
        in_maps.append({"inp": Wc})
    return in_maps


def kernel(**inputs):
    nc = _get_nc()
    in_maps = build_in_maps(inputs)
    B = np.asarray(inputs["x"]).shape[0]
    res = bass_utils.run_bass_kernel_spmd(nc, in_maps, core_ids=list(range(8)))
    outs = []
    for b in range(B):
        om = res.results[b]["out"]
        outs.append(om.T.reshape(-1)[:NC])
    return np.stack(outs).astype(np.float32)


# revision 16
# speedup vs baseline: 205.3483x; 1.1861x over previous
"""Trainium2 Bass kernel for nn_DeepTropNet (dense tropical transformer).

Strategy:
- Batch-parallel across cores (B=4 -> cores 0..3; cores 4..7 duplicate).
- Residual stream kept TRANSPOSED in SBUF: hT [D=128 partitions, L=197 free],
  so every projection is a natural TensorE matmul (contraction on partitions).
- All tropical (max-plus) contractions use the log-sum-exp trick at low
  temperature: max_i(a_i+b_i) = T*log(sum_i e^{a_i/T} e^{b_i/T}), separable ->
  a TensorE matmul of elementwise exponentials. Stabilizers are data-derived
  maxima (host-side for weights, on-device for activations).
- All weight transposes/exp-tables/bias-folds are host-side numpy prep.
- Custom DVE ops (affine_then_max/min) fuse the piecewise-linear fold.
"""
import sys

sys.path.insert(0, "/opt/trn_rl_repo")

import numpy as np

import concourse.bass as bass
import concourse.tile as tile
from concourse import bacc, mybir
from concourse import bass_utils

FP = mybir.dt.float32
AX = mybir.AxisListType
OP = mybir.AluOpType
ACTF = mybir.ActivationFunctionType

NL, D, H, DK, L, F, P, NC, PS = 2, 128, 8, 16, 197, 256, 8, 1000, 16
EPS = 1e-5
SCALE = DK ** -0.5
T1 = 0.01        # temperature for D=128-contraction tropical matmuls (Q,K,z)
T2 = 0.06        # temperature for DK=16-contraction tropical scores
NPATCH = 196
KCH = 6          # 768/128 contraction chunks for patch embed
KT0, KT1 = 128, L - 128   # score k-tiles


# ---------------------------------------------------------------- custom DVE ops
def _make_op(name, body_fn, reference):
    from concourse.dve_spec import Spec, lower, _has_src1
    from concourse.dve_uop import DveOpSpec
    import concourse.dve_ops as dve_ops

    for o in dve_ops.OPS:
        if o.name == name:
            return o
    spec = Spec(body=body_fn(), reference=reference)
    row = dve_ops._CUSTOM_DVE_ROW_BASE + len(dve_ops.OPS)
    assert row < 0x20
    dve_ops._SUB_OPCODE_FOR_NAME[name] = row
    shas = {}
    for ver in ("v3", "v4"):
        try:
            uops = lower(spec, ver=ver)
            shas[ver] = DveOpSpec(name=name, opcode=row, uops=uops,
                                  rd1_en=_has_src1(spec)).sha(ver)
        except Exception:
            pass
    op = dve_ops.DveOp(name, spec, subdim=False, uops_sha=shas)
    dve_ops.OPS.append(op)
    dve_ops.CUSTOM_DVE_SPECS[name] = spec
    return op


def _register_ops():
    from concourse.dve_spec import C0, C1, Src0, Src1, maxx, minn

    aff_max = _make_op(
        "ANT_AFFINE_THEN_MAX",
        lambda: maxx(Src0 * C0 + C1, Src1),
        lambda in0, in1, s0, s1, imm2: np.maximum(
            (in0.astype(np.float32) * s0 + s1), in1).astype(np.float32),
    )
    aff_min = _make_op(
        "ANT_AFFINE_THEN_MIN",
        lambda: minn(Src0 * C0 + C1, Src1),
        lambda in0, in1, s0, s1, imm2: np.minimum(
            (in0.astype(np.float32) * s0 + s1), in1).astype(np.float32),
    )
    return aff_max, aff_min


AFF_MAX, AFF_MIN = _register_ops()


# Bind Exp and Ln to the one ACT table set that holds both, so the score
# loop's Ln/Exp alternation doesn't reload tables (~2.7us per switch).
def _patch_act_tables():
    import concourse.hw_specs as hw_specs
    import concourse.bacc as bacc_mod
    if getattr(hw_specs, "_ant_combo_patched", False):
        return
    orig = hw_specs.get_activation_tables

    def patched(arch):
        tabs = orig(arch)
        A = mybir.ActivationFunctionType
        combo = "natural_log_exp_and_others"
        if combo in tabs and A.Exp in tabs[combo] and A.Ln in tabs[combo]:
            for name, fns in tabs.items():
                if name != combo:
                    fns.discard(A.Exp)
                    fns.discard(A.Ln)
        return tabs

    hw_specs.get_activation_tables = patched
    bacc_mod.get_activation_tables = patched
    hw_specs._ant_combo_patched = True


_patch_act_tables()


# ---------------------------------------------------------------- host-side prep
class _Pack:
    def __init__(self):
        self.cols = {}
        self.n = 0

    def add(self, name, ncols):
        self.cols[name] = (self.n, ncols)
        self.n += ncols

    def sl(self, name):
        return self.cols[name]


def _layout():
    pk = _Pack()
    pk.add("xp", KCH * NPATCH)        # per-batch patch data chunks
    pk.add("b0", L)
    pk.add("pw", KCH * 128)
    pk.add("onescol", 1)
    pk.add("meancol", 1)              # 1/128
    pk.add("e8", 8 * 8)               # unit-column blocks for sigma matmuls
    pk.add("segg", 2 * 128)           # per-group head segment lhsT (rows 0:8)
    pk.add("oneh", 8 * 128)           # row-h all-ones lhsT blocks (rows 0:8)
    pk.add("onesrow", 128)            # all ones; row 0 used as [1,128] lhsT
    pk.add("epscol", 1)
    pk.add("clampcol", 1)
    pk.add("zerocol", 1)
    pk.add("padneg", 1)
    for i in range(NL):
        s = f"_{i}"
        pk.add("ln1wb" + s, 2)
        pk.add("ln2wb" + s, 2)
        pk.add("wqexpT" + s, 256)
        pk.add("cq" + s, 2)
        pk.add("wkexpT" + s, 256)
        pk.add("ck" + s, 2)
        pk.add("wvT" + s, 256)
        pk.add("gwT" + s, 8)
        pk.add("gb8" + s, 1)          # rows 0:8
        pk.add("st8" + s, 1)          # rows 0:8: scale*T2/temp_h
        pk.add("svec" + s, 2)         # per-group scale/temp cols (pad rows 0)
        pk.add("woT" + s, 256)
        pk.add("bo2" + s, 1)
        pk.add("tuexpT" + s, F)
        pk.add("ctu" + s, 2)
        pk.add("lfa" + s, 16)         # col t*8+p
        pk.add("lfc" + s, 16)
        pk.add("glc" + s, 2)
        pk.add("glc1m" + s, 2)
        pk.add("cuT" + s, F)
        pk.add("cub" + s, 2)
        pk.add("fgT" + s, F)
        pk.add("fgb" + s, 2)
        pk.add("dnT" + s, 2 * 128)
        pk.add("dnb" + s, 1)
    pk.add("fnwb", 2)
    pk.add("hwT", 1024)
    pk.add("hb", 8)
    return pk


PK = _layout()
NCOL = PK.n


def _prep_pack(inp):
    W = np.zeros((128, NCOL), np.float32)

    def put(name, arr):
        o, n = PK.sl(name)
        a = np.asarray(arr, np.float32)
        assert a.ndim == 2 and a.shape[1] == n, (name, a.shape, n)
        W[: a.shape[0], o:o + n] = a

    bn_s = inp["bn_gamma"] / (inp["bn_run_range"] + EPS)
    bn_b = inp["bn_beta"] - inp["bn_run_max"] * bn_s
    pos = inp["pos_embed"][0]                                     # [L, D]
    clsb = inp["cls_token"].reshape(D)
    b0 = np.empty((D, L), np.float32)
    b0[:, 0] = bn_s * (clsb + pos[0]) + bn_b
    b0[:, 1:] = (bn_s[:, None] * (inp["patch_b"][:, None] + pos[1:].T)
                 + bn_b[:, None])
    put("b0", b0)

    pwT = (bn_s[:, None] * inp["patch_w"]).T                      # [768, D]
    put("pw", np.concatenate([pwT[128 * k:128 * (k + 1)] for k in range(KCH)], 1))

    put("onescol", np.ones((128, 1)))
    put("meancol", np.full((128, 1), 1.0 / D))
    e8 = np.zeros((128, 64), np.float32)
    for h in range(H):
        e8[:, 8 * h + h] = 1.0
    put("e8", e8)
    segg = np.zeros((8, 2 * 128), np.float32)
    for g in range(2):
        for j in range(4):
            segg[4 * g + j, 128 * g + 32 * j:128 * g + 32 * j + DK] = 1.0
    put("segg", segg)
    oneh = np.zeros((8, 8 * 128), np.float32)
    for h in range(H):
        oneh[h, 128 * h:128 * (h + 1)] = 1.0
    put("oneh", oneh)
    put("onesrow", np.ones((128, 128)))
    put("epscol", np.full((128, 1), EPS))
    put("clampcol", np.full((128, 1), 1e-30))
    put("zerocol", np.zeros((128, 1)))
    padneg = np.zeros((128, 1), np.float32)
    for j in range(4):
        padneg[32 * j + DK:32 * (j + 1)] = -1e5
    put("padneg", padneg)

    def pad32_rows(vec):
        # [D] head-indexed (16h+d) -> two [128] group columns at rows 32j+d
        out = np.zeros((2, 128), np.float32)
        for h in range(H):
            g, j = divmod(h, 4)
            out[g, 32 * j:32 * j + DK] = vec[DK * h:DK * (h + 1)]
        return out

    for i in range(NL):
        s = f"_{i}"
        put("ln1wb" + s, np.stack([inp["n1_w"][i], inp["n1_b"][i]], 1))
        put("ln2wb" + s, np.stack([inp["n2_w"][i], inp["n2_b"][i]], 1))
        for nm, bkey in (("wq", "bq"), ("wk", "bk")):
            Wt = inp[nm][i]
            mW = Wt.max(1)
            we = np.exp((Wt - mW[:, None]) / T1).T          # [i, o=16h+d]
            wep = np.zeros((128, 256), np.float32)
            for h in range(H):
                g, j = divmod(h, 4)
                wep[:, 128 * g + 32 * j:128 * g + 32 * j + DK] = \
                    we[:, DK * h:DK * (h + 1)]
            put(nm + "expT" + s, wep)
            put("c" + nm[1] + s, pad32_rows(mW + inp[bkey][i]).T)
        wvp = np.zeros((128, 256), np.float32)
        wvT_ = inp["wv"][i].T                               # [i, 16h+d]
        for h in range(H):
            g, j = divmod(h, 4)
            wvp[:, 128 * g + 32 * j:128 * g + 32 * j + DK] = \
                wvT_[:, DK * h:DK * (h + 1)]
        put("wvT" + s, wvp)
        put("gwT" + s, inp["gate_w"][i].T)
        put("gb8" + s, inp["gate_b"][i][:, None])
        put("st8" + s, (SCALE * T2 / inp["temp"][i])[:, None])
        put("svec" + s, pad32_rows(np.repeat(SCALE / inp["temp"][i], DK)).T)
        woT_ = inp["wo"][i].T                               # [dD=16h+d, o2]
        wop = np.zeros((128, 256), np.float32)
        for h in range(H):
            g, j = divmod(h, 4)
            wop[32 * j:32 * j + DK, 128 * g:128 * (g + 1)] = \
                woT_[DK * h:DK * (h + 1), :]
        put("woT" + s, wop)
        put("bo2" + s, (inp["bo"][i] + inp["wo"][i] @ inp["bv"][i])[:, None])
        tu = inp["tu_w"][i]
        mtu = tu.max(1)
        put("tuexpT" + s, np.exp((tu - mtu[:, None]) / T1).T)
        ctu = mtu + inp["tu_b"][i]
        put("ctu" + s, np.stack([ctu[:128], ctu[128:]], 1))
        lfa = np.zeros((128, 16), np.float32)
        lfc = np.zeros((128, 16), np.float32)
        for t in range(2):
            for p in range(P):
                lfa[:, t * 8 + p] = inp["lf_a"][i][p, 128 * t:128 * (t + 1)]
                lfc[:, t * 8 + p] = inp["lf_c"][i][p, 128 * t:128 * (t + 1)]
        put("lfa" + s, lfa)
        put("lfc" + s, lfc)
        gl = 1.0 / (1.0 + np.exp(-inp["lf_gate"][i]))
        put("glc" + s, np.stack([gl[:128], gl[128:]], 1))
        put("glc1m" + s, np.stack([1 - gl[:128], 1 - gl[128:]], 1))
        put("cuT" + s, inp["cu_w"][i].T)
        put("cub" + s, np.stack([inp["cu_b"][i][:128], inp["cu_b"][i][128:]], 1))
        put("fgT" + s, inp["fg_w"][i].T)
        put("fgb" + s, np.stack([inp["fg_b"][i][:128], inp["fg_b"][i][128:]], 1))
        dnT = inp["dn_w"][i].T                                    # [F, D]
        put("dnT" + s, np.concatenate([dnT[:128], dnT[128:]], 1))
        put("dnb" + s, inp["dn_b"][i][:, None])

    put("fnwb", np.stack([inp["fn_w"], inp["fn_b"]], 1))
    hwT = np.zeros((128, 1024), np.float32)
    hb = np.zeros((128, 8), np.float32)
    hw_pad = np.zeros((1024, D), np.float32)
    hw_pad[:NC] = inp["head_w"]
    hb_pad = np.zeros(1024, np.float32)
    hb_pad[:NC] = inp["head_b"]
    for j in range(8):
        hwT[:, 128 * j:128 * (j + 1)] = hw_pad[128 * j:128 * (j + 1)].T
        hb[:, j] = hb_pad[128 * j:128 * (j + 1)]
    put("hwT", hwT)
    put("hb", hb)
    return W


def _prep_x(inp, b):
    xb = inp["x"][b]
    xp = xb.reshape(3, 14, PS, 14, PS).transpose(1, 3, 0, 2, 4).reshape(
        NPATCH, 3 * PS * PS)
    xpT = np.ascontiguousarray(xp.T.astype(np.float32))
    return np.concatenate([xpT[128 * k:128 * (k + 1)] for k in range(KCH)], 1)


# ---------------------------------------------------------------- bass program
def _build_program(reps=1):
    nc = bacc.Bacc("TRN2", target_bir_lowering=False, debug=False,
                   enable_asserts=True, num_devices=8)
    inp_d = nc.dram_tensor("inp", [128, NCOL], FP, kind="ExternalInput").ap()
    out_d = nc.dram_tensor("out", [128, 8], FP, kind="ExternalOutput").ap()
    with tile.TileContext(nc) as tc:
        _bass_body(nc, tc, inp_d, out_d, reps=reps)
    nc.compile()
    return nc


def _bass_body(nc, tc, inp_d, out_d, reps=1):
    import contextlib
    ctx = contextlib.ExitStack()
    perm = ctx.enter_context(tc.tile_pool(name="perm", bufs=1))
    work = ctx.enter_context(tc.tile_pool(name="work", bufs=2))
    psp = ctx.enter_context(tc.tile_pool(name="psp", bufs=1, space="PSUM"))

    WPK = perm.tile([128, NCOL], FP)
    nc.sync.dma_start(WPK[:], inp_d[:])

    def wp(name, rows=128):
        o, n = PK.sl(name)
        return WPK[0:rows, o:o + n]

    def col(name, j=0, rows=128):
        o, n = PK.sl(name)
        return WPK[0:rows, o + j:o + j + 1]

    _pp_ctr = [0]

    def pp(shape, tag="pp", bufs=3):
        _pp_ctr[0] += 1
        return psp.tile(shape, FP, tag=tag, bufs=bufs,
                        name=f"{tag}{_pp_ctr[0]}",
                        padded_shape=[128, 512])

    onesrow = wp("onesrow")[0:1, :]
    meancol = wp("meancol")

    hT = perm.tile([128, 2 * L], FP)     # residual cols 0:197, x^2 scratch 197:394

    if reps > 1:
        loop_cm = tc.For_i(0, reps, 1)
        loop_cm.__enter__()

    # ---- patch embed ----
    pe = pp([128, NPATCH])
    xo, _ = PK.sl("xp")
    po, _ = PK.sl("pw")
    for k in range(KCH):
        nc.tensor.matmul(pe[:], WPK[:, po + 128 * k: po + 128 * (k + 1)],
                         WPK[:, xo + NPATCH * k: xo + NPATCH * (k + 1)],
                         start=(k == 0), stop=(k == KCH - 1))
    nc.vector.tensor_tensor(hT[:, 1:L], pe[:], wp("b0")[:, 1:L], op=OP.add)
    nc.vector.tensor_copy(hT[:, 0:1], wp("b0")[:, 0:1])

    def layer_norm(wb_ap):
        sq = hT[:, L:2 * L]
        nc.vector.tensor_tensor(sq, hT[:, 0:L], hT[:, 0:L], op=OP.mult)
        stats = pp([1, 2 * L])
        nc.tensor.matmul(stats[:], meancol, hT[:], start=True, stop=True)
        mean = work.tile([1, L], FP, tag="mean")
        nc.vector.tensor_copy(mean[:], stats[0:1, 0:L])
        msq = work.tile([1, L], FP, tag="msq")
        nc.vector.tensor_tensor(msq[:], mean[:], mean[:], op=OP.mult)
        var = work.tile([1, L], FP, tag="var")
        nc.vector.tensor_tensor(var[:], stats[0:1, L:2 * L], msq[:],
                                op=OP.subtract)
        std = work.tile([1, L], FP, tag="std")
        nc.scalar.activation(std[:], var[:], ACTF.Sqrt, bias=col("epscol", rows=1))
        rstd = work.tile([1, L], FP, tag="rstd")
        nc.vector.reciprocal_approx_fast(out=rstd[:], in_=std[:])
        mr = work.tile([1, L], FP, tag="mr")
        nc.vector.tensor_tensor(mr[:], mean[:], rstd[:], op=OP.mult)
        rstdB = pp([128, L])
        nc.tensor.matmul(rstdB[:], onesrow, rstd[:], start=True, stop=True)
        mrB = pp([128, L])
        nc.tensor.matmul(mrB[:], onesrow, mr[:], start=True, stop=True)
        t1_ = work.tile([128, L], FP, tag="lnt1")
        nc.vector.tensor_tensor(t1_[:], hT[:, 0:L], rstdB[:], op=OP.mult)
        t2_ = work.tile([128, L], FP, tag="lnt2")
        nc.vector.tensor_tensor(t2_[:], t1_[:], mrB[:], op=OP.subtract)
        hn = work.tile([128, L], FP, tag="hn")
        nc.vector.tensor_scalar(hn[:], t2_[:], wb_ap[:, 0:1], wb_ap[:, 1:2],
                                op0=OP.mult, op1=OP.add)
        return hn

    def trop_exp_rhs(hn, mxB):
        from concourse import bass_isa
        nc.gpsimd.partition_all_reduce(mxB[:], hn[:], channels=128,
                                       reduce_op=bass_isa.ReduceOp.max)
        xc = work.tile([128, L], FP, tag="xc")
        nc.vector.tensor_tensor(xc[:], hn[:], mxB[:], op=OP.subtract)
        xe = work.tile([128, L], FP, tag="xe")
        nc.scalar.activation(xe[:], xc[:], ACTF.Exp, scale=1.0 / T1, bias=col("zerocol"))
        return xe

    def trop_project(xe, mxB, lhsT, cvec, out_tile, M=128):
        sp = pp([128, L])
        nc.tensor.matmul(sp[0:M, :], lhsT, xe[:], start=True, stop=True)
        lg = work.tile([128, L], FP, tag="trop_lg")
        nc.scalar.activation(lg[0:M, :], sp[0:M, :], ACTF.Ln, bias=col("clampcol", rows=M))
        nc.vector.tensor_scalar(out_tile[0:M, :], lg[0:M, :], T1, cvec,
                                op0=OP.mult, op1=OP.add)
        nc.vector.tensor_tensor(out_tile[0:M, :], out_tile[0:M, :],
                                mxB[0:M, :], op=OP.add)

    def global_max_exp(src, tag):
        fm = work.tile([128, 1], FP, tag=tag + "fm")
        nc.vector.tensor_reduce(fm[:], src[:], axis=AX.X, op=OP.max)
        gm = work.tile([1, 1], FP, tag=tag + "gm")
        nc.gpsimd.tensor_reduce(gm[:], fm[:], axis=AX.C, op=OP.max)
        gmB = pp([128, 1])
        nc.tensor.matmul(gmB[:], onesrow, gm[:], start=True, stop=True)
        nb = work.tile([128, 1], FP, tag=tag + "nb")
        nc.vector.tensor_scalar(nb[:], gmB[:], -1.0 / T2, None, op0=OP.mult)
        ex = work.tile([128, L], FP, tag=tag + "ex")
        nc.scalar.activation(ex[:], src[:], ACTF.Exp, bias=nb[:], scale=1.0 / T2)
        return ex

    for i in range(NL):
        s = f"_{i}"
        hn = layer_norm(wp("ln1wb" + s))
        mxB = work.tile([128, L], FP, tag="mxB", name=f"mxA{i}")
        xe = trop_exp_rhs(hn, mxB)

        # Q/K tropical projections into 32-padded head layout: [128, 2L],
        # block g holds heads 4g..4g+3 at partition groups 32j (+16 zero rows)
        Qt = work.tile([128, 2 * L], FP, tag="Qt")
        Kt = work.tile([128, 2 * L], FP, tag="Kt")
        for g in range(2):
            trop_project(xe, mxB, wp("wqexpT" + s)[:, 128 * g:128 * (g + 1)],
                         col("cq" + s, j=g), Qt[:, L * g:L * (g + 1)])
            trop_project(xe, mxB, wp("wkexpT" + s)[:, 128 * g:128 * (g + 1)],
                         col("ck" + s, j=g), Kt[:, L * g:L * (g + 1)])

        # V in the same padded layout: Vsb[0:kn, 256t+128g+32j : +16] = V head
        Vsb = work.tile([128, 512], FP, tag="Vsb")
        for t, (k0, kn) in enumerate(((0, KT0), (KT0, KT1))):
            for g in range(2):
                vp = pp([128, 128])
                nc.tensor.matmul(vp[0:kn, :], hn[:, k0:k0 + kn],
                                 wp("wvT" + s)[:, 128 * g:128 * (g + 1)],
                                 start=True, stop=True)
                nc.vector.tensor_copy(
                    Vsb[0:kn, 256 * t + 128 * g:256 * t + 128 * (g + 1)],
                    vp[0:kn, :])

        gp = pp([8, L])
        nc.tensor.matmul(gp[:], wp("gwT" + s), hn[:], start=True, stop=True)
        gsig = work.tile([8, L], FP, tag="gsig")
        nc.scalar.activation(gsig[:], gp[:], ACTF.Sigmoid,
                             bias=col("gb8" + s, rows=8))
        gts = work.tile([8, L], FP, tag="gts")
        nc.vector.tensor_scalar(gts[:], gsig[:], col("st8" + s, rows=8), None,
                                op0=OP.mult)

        # Qcs = (1 - g_seg) * Qt * svec  (svec zero on pad rows)
        gsegB = pp([128, 2 * L])
        for g in range(2):
            nc.tensor.matmul(gsegB[:, L * g:L * (g + 1)],
                             wp("segg", rows=8)[:, 128 * g:128 * (g + 1)],
                             gsig[:], start=True, stop=True)
        qts = work.tile([128, 2 * L], FP, tag="qts")
        nc.vector.tensor_tensor(qts[:], Qt[:], gsegB[:], op=OP.mult)
        qd = work.tile([128, 2 * L], FP, tag="qd")
        nc.vector.tensor_tensor(qd[:], Qt[:], qts[:], op=OP.subtract)
        Qcs = work.tile([128, 2 * L], FP, tag="Qcs")
        for g in range(2):
            nc.vector.tensor_scalar(Qcs[:, L * g:L * (g + 1)],
                                    qd[:, L * g:L * (g + 1)],
                                    col("svec" + s, j=g), None, op0=OP.mult)

        # exp((Qt - gmax)/T2) with pad rows forced to ~0 via padneg bias
        def gmax_exp(srcT, tag):
            from concourse import bass_isa
            fm = work.tile([128, 1], FP, tag=tag + "fm")
            nc.vector.tensor_reduce(fm[:], srcT[:], axis=AX.X, op=OP.max)
            gm = work.tile([128, 1], FP, tag=tag + "gm")
            nc.gpsimd.partition_all_reduce(gm[:], fm[:], channels=128,
                                           reduce_op=bass_isa.ReduceOp.max)
            nb = work.tile([128, 1], FP, tag=tag + "nb")
            nc.vector.tensor_scalar(nb[:], gm[:], -1.0 / T2, col("padneg"),
                                    op0=OP.mult, op1=OP.add)
            ex = work.tile([128, 2 * L], FP, tag=tag + "ex")
            nc.scalar.activation(ex[:], srcT[:], ACTF.Exp, bias=nb[:],
                                 scale=1.0 / T2)
            return ex

        Qe2 = gmax_exp(Qt, "q2")
        Ke2 = gmax_exp(Kt, "k2")

        sig8 = psp.tile([8, L], FP, tag="sig8", padded_shape=[128, 512])
        eo, _ = PK.sl("e8")
        oSums = []
        for g in range(2):
            oA = psp.tile([128, L], FP, tag="oA", padded_shape=[128, 512], name=f"oA{i}{g}")
            oB = psp.tile([128, L], FP, tag="oB", padded_shape=[128, 512], name=f"oB{i}{g}")
            for j in range(4):
                h = 4 * g + j
                ps32 = slice(32 * j, 32 * (j + 1))
                gB = pp([128, L])
                nc.tensor.matmul(gB[:], wp("oneh", rows=8)[:, 128 * h:128 * (h + 1)],
                                 gts[:], start=True, stop=True)
                for t, (k0, kn) in enumerate(((0, KT0), (KT0, KT1))):
                    sts = pp([128, L])
                    nc.tensor.matmul(sts[0:kn, :],
                                     Ke2[ps32, L * g + k0:L * g + k0 + kn],
                                     Qe2[ps32, L * g:L * (g + 1)],
                                     start=True, stop=True,
                                     tile_position=(32 * j, 0))
                    scs = pp([128, L])
                    nc.tensor.matmul(scs[0:kn, :],
                                     Kt[ps32, L * g + k0:L * g + k0 + kn],
                                     Qcs[ps32, L * g:L * (g + 1)],
                                     start=True, stop=True,
                                     tile_position=(32 * j, 0))
                    lg = work.tile([128, L], FP, tag="sc_lg")
                    nc.scalar.activation(lg[0:kn, :], sts[0:kn, :], ACTF.Ln,
                                         bias=col("clampcol", rows=kn))
                    u = work.tile([128, L], FP, tag="sc_u")
                    nc.vector.tensor_tensor(u[0:kn, :], lg[0:kn, :],
                                            gB[0:kn, :], op=OP.mult)
                    u2 = work.tile([128, L], FP, tag="sc_u2")
                    nc.vector.tensor_tensor(u2[0:kn, :], u[0:kn, :],
                                            scs[0:kn, :], op=OP.add)
                    Pt = work.tile([128, L], FP, tag="sc_P")
                    nc.scalar.activation(Pt[0:kn, :], u2[0:kn, :], ACTF.Exp,
                                         bias=col("zerocol", rows=kn))
                    first = (h == 0 and t == 0)
                    last = (h == H - 1 and t == 1)
                    nc.tensor.matmul(
                        sig8[:], WPK[0:kn, eo + 8 * h:eo + 8 * h + 8],
                        Pt[0:kn, :], start=first, stop=last)
                    ot = oA if t == 0 else oB
                    nc.tensor.matmul(
                        ot[ps32, :],
                        Vsb[0:kn, 256 * t + 128 * g + 32 * j:
                            256 * t + 128 * g + 32 * (j + 1)],
                        Pt[0:kn, :], start=True, stop=True,
                        tile_position=(0, 32 * j))
            oSum = work.tile([128, L], FP, tag="oSum", name=f"oSum{i}{g}")
            nc.vector.tensor_copy(oSum[:], oA[:])
            nc.vector.tensor_tensor(oSum[:], oSum[:], oB[:], op=OP.add)
            oSums.append(oSum)
        rs8 = work.tile([8, L], FP, tag="rs8")
        nc.vector.reciprocal_approx_fast(out=rs8[:], in_=sig8[:])
        pj = pp([128, L])
        for g in range(2):
            rsB = pp([128, L])
            nc.tensor.matmul(rsB[:], wp("segg", rows=8)[:, 128 * g:128 * (g + 1)],
                             rs8[:], start=True, stop=True)
            onrm = work.tile([128, L], FP, tag="onrm")
            nc.vector.tensor_tensor(onrm[:], oSums[g][:], rsB[:], op=OP.mult)
            nc.tensor.matmul(pj[:], wp("woT" + s)[:, 128 * g:128 * (g + 1)],
                             onrm[:], start=(g == 0), stop=(g == 1))
        nc.vector.scalar_tensor_tensor(hT[:, 0:L], pj[:], col("bo2" + s),
                                       hT[:, 0:L], op0=OP.add, op1=OP.add)

        # ---- FFN ----
        hn2 = layer_norm(wp("ln2wb" + s))
        mxB2 = work.tile([128, L], FP, tag="mxB", name=f"mxF{i}")
        xe2 = trop_exp_rhs(hn2, mxB2)
        dp = psp.tile([128, L], FP, tag="dp", padded_shape=[128, 512])
        trops, clss, gfs = [], [], []
        for t in range(2):
            zT = work.tile([128, L], FP, tag="zT")
            trop_project(xe2, mxB2, wp("tuexpT" + s)[:, 128 * t:128 * (t + 1)],
                         col("ctu" + s, j=t), zT)
            zmx = work.tile([128, L], FP, tag="zmx")
            zmn = work.tile([128, L], FP, tag="zmn")
            nc.vector.tensor_scalar(zmx[:], zT[:], col("lfa" + s, j=t * 8),
                                    col("lfc" + s, j=t * 8), op0=OP.mult,
                                    op1=OP.add)
            nc.vector.tensor_copy(zmn[:], zmx[:])
            for p in range(1, P):
                nc.vector._custom_dve(AFF_MAX, out=zmx[:], in0=zT[:],
                                      in1=zmx[:],
                                      s0=col("lfa" + s, j=t * 8 + p),
                                      s1=col("lfc" + s, j=t * 8 + p))
                nc.vector._custom_dve(AFF_MIN, out=zmn[:], in0=zT[:],
                                      in1=zmn[:],
                                      s0=col("lfa" + s, j=t * 8 + p),
                                      s1=col("lfc" + s, j=t * 8 + p))
            trop_t = work.tile([128, L], FP, tag="trop_t", name=f"trop{i}{t}")
            nc.vector.tensor_scalar(trop_t[:], zmx[:], col("glc" + s, j=t),
                                    None, op0=OP.mult)
            nc.vector.scalar_tensor_tensor(trop_t[:], zmn[:],
                                           col("glc1m" + s, j=t), trop_t[:],
                                           op0=OP.mult, op1=OP.add)
            trops.append(trop_t)
        for t in range(2):
            cp = pp([128, L])
            nc.tensor.matmul(cp[:], wp("cuT" + s)[:, 128 * t:128 * (t + 1)],
                             hn2[:], start=True, stop=True)
            cls_t = work.tile([128, L], FP, tag="cls_t", name=f"cls{i}{t}")
            nc.scalar.activation(cls_t[:], cp[:], ACTF.Gelu,
                                 bias=col("cub" + s, j=t))
            clss.append(cls_t)
        for t in range(2):
            fgp = pp([128, L])
            nc.tensor.matmul(fgp[:], wp("fgT" + s)[:, 128 * t:128 * (t + 1)],
                             hn2[:], start=True, stop=True)
            gf = work.tile([128, L], FP, tag="gf", name=f"gf{i}{t}")
            nc.scalar.activation(gf[:], fgp[:], ACTF.Sigmoid,
                                 bias=col("fgb" + s, j=t))
            gfs.append(gf)
        for t in range(2):
            dt_ = work.tile([128, L], FP, tag="dt_")
            nc.vector.tensor_tensor(dt_[:], trops[t][:], clss[t][:],
                                    op=OP.subtract)
            fused = work.tile([128, L], FP, tag="fused")
            nc.vector.tensor_tensor(fused[:], gfs[t][:], dt_[:], op=OP.mult)
            nc.vector.tensor_tensor(fused[:], fused[:], clss[t][:], op=OP.add)
            nc.tensor.matmul(dp[:], wp("dnT" + s)[:, 128 * t:128 * (t + 1)],
                             fused[:], start=(t == 0), stop=(t == 1))
        nc.vector.scalar_tensor_tensor(hT[:, 0:L], dp[:], col("dnb" + s),
                                       hT[:, 0:L], op0=OP.add, op1=OP.add)

    # ---- final LN (cls column only) + head ----
    h0 = work.tile([128, 1], FP, tag="h0")
    nc.vector.tensor_copy(h0[:], hT[:, 0:1])
    sq0 = work.tile([128, 1], FP, tag="sq0")
    nc.vector.tensor_tensor(sq0[:], h0[:], h0[:], op=OP.mult)
    st0 = pp([1, 2])
    nc.tensor.matmul(st0[0:1, 0:1], meancol, h0[:], start=True, stop=True)
    nc.tensor.matmul(st0[0:1, 1:2], meancol, sq0[:], start=True, stop=True)
    mean0 = work.tile([1, 2], FP, tag="mean0")
    nc.vector.tensor_copy(mean0[:], st0[0:1, 0:2])
    var0 = work.tile([1, 1], FP, tag="var0")
    nc.vector.tensor_tensor(var0[:], mean0[0:1, 0:1], mean0[0:1, 0:1],
                            op=OP.mult)
    nc.vector.tensor_tensor(var0[:], mean0[0:1, 1:2], var0[:], op=OP.subtract)
    std0 = work.tile([1, 1], FP, tag="std0")
    nc.scalar.activation(std0[:], var0[:], ACTF.Sqrt, bias=col("epscol", rows=1))
    rstd0 = work.tile([1, 1], FP, tag="rstd0")
    nc.vector.reciprocal_approx_fast(out=rstd0[:], in_=std0[:])
    mrow = work.tile([1, 2], FP, tag="mrow")
    nc.vector.tensor_tensor(mrow[0:1, 0:1], mean0[0:1, 0:1], rstd0[:],
                            op=OP.mult)
    nc.vector.tensor_copy(mrow[0:1, 1:2], rstd0[:])
    mB = pp([128, 2])
    nc.tensor.matmul(mB[:], onesrow, mrow[:], start=True, stop=True)
    t0 = work.tile([128, 1], FP, tag="t0")
    nc.vector.tensor_tensor(t0[:], h0[:], mB[:, 1:2], op=OP.mult)
    nc.vector.tensor_tensor(t0[:], t0[:], mB[:, 0:1], op=OP.subtract)
    hf = work.tile([128, 1], FP, tag="hf")
    nc.vector.tensor_scalar(hf[:], t0[:], wp("fnwb")[:, 0:1],
                            wp("fnwb")[:, 1:2], op0=OP.mult, op1=OP.add)
    hd = pp([128, 8])
    for j in range(8):
        nc.tensor.matmul(hd[:, j:j + 1], wp("hwT")[:, 128 * j:128 * (j + 1)],
                         hf[:], start=True, stop=True)
    ob = work.tile([128, 8], FP, tag="ob")
    nc.vector.tensor_tensor(ob[:], hd[:], wp("hb"), op=OP.add)
    if reps > 1:
        loop_cm.__exit__(None, None, None)
    nc.sync.dma_start(out_d[:], ob[:])
    ctx.close()


# ---------------------------------------------------------------- entry point
_NC_CACHE = []


def _get_nc():
    if not _NC_CACHE:
        _NC_CACHE.append(_build_program())
    return _NC_CACHE[0]


def build_in_maps(inputs):
    inputs = {k: np.asarray(v, np.float32) for k, v in inputs.items()}
    Wsh = _prep_pack(inputs)
    B = inputs["x"].shape[0]
    in_maps = []
    o, n = PK.sl("xp")
    for c in range(8):
        Wc = Wsh.copy()
        Wc[:, o:o + n] = _prep_x(inputs, c % B)
        in_maps.append({"inp": Wc})
    return in_maps


def kernel(**inputs):
    nc = _get_nc()
    in_maps = build_in_maps(inputs)
    B = np.asarray(inputs["x"]).shape[0]
    res = bass_utils.run_bass_kernel_spmd(nc, in_maps, core_ids=list(range(8)))
    outs = []
    for b in range(B):
        om = res.results[b]["out"]
        outs.append(om.T.reshape(-1)[:NC])
    return np.stack(outs).astype(np.float32)
